# revision 1
# baseline (speedup 1.0000x reference)
"""DeepseekV3 MLA flash-attention prefill kernel for 8 Trainium2 NeuronCores.

Sharding strategy (SPMD, one program for all 8 cores):
  Stage A (sequence-parallel, feature-major): core c computes the low-rank
    down-projections q_a = rms_norm(X @ Wqa), c_kv = rms_norm(ckv[:, :512]),
    k_pe(roped) for its 256 rows directly in transposed layout
    (lhsT = weight chunks, rhs = X^T), then AllGathers them (kv first so
    stage B K/V work can overlap the q gather).
  Stage B (head-parallel): core c owns heads {2c, 2c+1}: all q projections
    (Wqb + RoPE) are precomputed per panel, then causal attention runs in
    (k, q) layout: softmax without max-subtraction, fully-masked k-blocks
    skipped, diagonal blocks masked with GpSimd affine_select, per-q
    normalization folded into the attn^T eviction.  Each head's attn^T is
    exchanged with its own AllToAll so the first overlaps the second head.
  Each core then computes its 256 output rows against the full Wo
    (weights preloaded into a disjoint SBUF region early). Host concatenates.
"""

import sys

if '/opt/trn_rl_repo' not in sys.path:
    sys.path.insert(0, '/opt/trn_rl_repo')

import numpy as np
import ml_dtypes

import concourse.bass as bass
import concourse.mybir as mybir
import concourse.tile as tile
from concourse import bacc
from concourse.bass_utils import run_bass_kernel_spmd

f32 = mybir.dt.float32
f32r = mybir.dt.float32r
bf16 = mybir.dt.bfloat16
i32 = mybir.dt.int32
AF = mybir.ActivationFunctionType
ALU = mybir.AluOpType

NC_ = 8            # cores
S = 2048           # sequence
HID = 2048
QLR = 1536         # q lora rank
KVLR = 512         # kv lora rank
ROPE = 64
NOPE = 128
VD = 128
NH = 16
HPC = NH // NC_    # heads per core = 2
SL = S // NC_      # rows per core = 256
PANEL = 512        # q panel width
NPANEL = S // PANEL
NKB = S // 128     # 16 k blocks
QCH = QLR // 128   # 12
KCH = KVLR // 128  # 4
HCH = HID // 128   # 16
THETA = 10000.0
SM_SCALE = float((NOPE + ROPE) ** -0.5)
PI = float(np.pi)

DT = bf16          # matmul dtype: bf16 or f32r

_CACHE = {}


def _range_reduce_sin(nc, pool, src_ap, P, W, bias, name, tag):
    """sin(src + bias) with range reduction to [-pi, pi]. src may be PSUM."""
    t0 = pool.tile([P, W], f32, name=f"{name}_t0", tag="rr0", bufs=1)
    ti = pool.tile([P, W], i32, name=f"{name}_ti", tag="rr1", bufs=1)
    tf = pool.tile([P, W], f32, name=f"{name}_tf", tag="rr2", bufs=1)
    arg = pool.tile([P, W], f32, name=f"{name}_arg", tag="rr3", bufs=1)
    res = pool.tile([P, W], f32, name=f"{name}_sin", tag=tag, bufs=2)
    nc.vector.tensor_scalar(out=t0[:], in0=src_ap, scalar1=bias, scalar2=None, op0=ALU.add)
    nc.vector.tensor_scalar(out=tf[:], in0=t0[:], scalar1=1.0 / (2 * PI), scalar2=None, op0=ALU.mult)
    nc.vector.tensor_copy(ti[:], tf[:])
    nc.vector.tensor_copy(tf[:], ti[:])
    nc.vector.scalar_tensor_tensor(out=arg[:], in0=tf[:], scalar=-2 * PI, in1=t0[:], op0=ALU.mult, op1=ALU.add)
    nc.scalar.activation(res[:], arg[:], AF.Sin)
    return res


def build_program(dt):
    nc = bacc.Bacc("TRN2", target_bir_lowering=False, debug=False, num_devices=NC_)

    def din(name, shape):
        return nc.dram_tensor(name, shape, dt, kind="ExternalInput")

    # ---- external I/O (per-core data) ----
    x_t = din("x_t", [HID, SL])                 # X rows, transposed (hid-major)
    pos = nc.dram_tensor("pos", [1, SL], f32, kind="ExternalInput")
    pos_all = nc.dram_tensor("pos_all", [1, S], f32, kind="ExternalInput")
    wa = din("wa", [HID, QLR + KVLR + ROPE])    # [Wqa | Wkva(kv) | Wkva(pe, deint)]
    wqb = din("wqb", [QLR, HPC * 256])          # [nope|pe_d|rot] per head
    wkvb_k = din("wkvb_k", [KVLR, HPC * NOPE])
    wkvb_v = din("wkvb_v", [KVLR, HPC * VD])
    wo = din("wo", [NH * VD, HID])
    ones_col = din("ones_col", [128, 1])
    ones_row = nc.dram_tensor("ones_row", [1, 128], f32, kind="ExternalInput")
    invf_col = nc.dram_tensor("invf_col", [ROPE, 1], f32, kind="ExternalInput")
    out_loc = nc.dram_tensor("out_loc", [SL, HID], f32, kind="ExternalOutput")

    NAG_KV = KCH + 1
    WAW = QLR + KVLR + ROPE  # 2112

    with tile.TileContext(nc) as tc:
        with tc.tile_pool(name="dram", bufs=1, space="DRAM") as dpool, \
             tc.tile_pool(name="consts", bufs=1) as cpool:
            ag_in_kv = dpool.tile([NAG_KV * 128, SL], dt)
            ag_out_kv = dpool.tile([NC_ * NAG_KV * 128, SL], dt, addr_space="Shared")
            ag_in_q = dpool.tile([QCH * 128, SL], dt)
            ag_out_q = dpool.tile([NC_ * QCH * 128, SL], dt, addr_space="Shared")
            a2a_in = [dpool.tile([NC_ * VD, SL], dt, name=f"a2a_in{h}") for h in range(HPC)]
            a2a_out = [dpool.tile([NC_ * VD, SL], dt, name=f"a2a_out{h}") for h in range(HPC)]

            ocol = cpool.tile([128, 1], dt)
            orow = cpool.tile([1, 128], f32r)
            invc_t = cpool.tile([ROPE, 1], f32)
            pos_all_t = cpool.tile([1, S], f32r)
            pos_t = cpool.tile([1, SL], f32r)
            nc.sync.dma_start(out=ocol[:], in_=ones_col[:])
            nc.sync.dma_start(out=orow[:], in_=ones_row[:].bitcast(f32r))
            nc.sync.dma_start(out=invc_t[:], in_=invf_col[:])
            nc.sync.dma_start(out=pos_all_t[:], in_=pos_all[:].bitcast(f32r))
            nc.sync.dma_start(out=pos_t[:], in_=pos[:].bitcast(f32r))

            wo_res = False
            wo_map = {}

            # ================= Stage A: transposed down projections =================
            with tc.tile_pool(name="sa_x", bufs=1) as xp, \
                 tc.tile_pool(name="sa_w", bufs=1) as wp, \
                 tc.tile_pool(name="sa_res", bufs=1) as rp, \
                 tc.tile_pool(name="sa_tmp", bufs=2) as tp, \
                 tc.tile_pool(name="sa_ps", bufs=2, space="PSUM") as pp, \
                 tc.tile_pool(name="sa_ps1", bufs=1, space="PSUM") as pp1:

                xts = []
                for k in range(HCH):
                    xt = xp.tile([128, SL], dt, name=f"xt{k}")
                    nc.sync.dma_start(out=xt[:], in_=x_t[128 * k:128 * (k + 1), :])
                    xts.append(xt)
                wa_res = []
                for hc in range(HCH):
                    wt = wp.tile([128, WAW], dt, name=f"wA_{hc}")
                    nc.sync.dma_start(out=wt[:], in_=wa[128 * hc:128 * (hc + 1), :])
                    wa_res.append(wt)

                def a_chunk(o, c0, width, tag):
                    """accumulate chunk [c0:c0+width] of the 2112-wide projection"""
                    ps = pp.tile([width, SL], f32, name=f"ps_{tag}_{o}", tag="a_ps", bufs=2)
                    for hc in range(HCH):
                        nc.tensor.matmul(ps[:], wa_res[hc][:, c0:c0 + width], xts[hc][:],
                                         start=(hc == 0), stop=(hc == HCH - 1))
                    return ps

                ssq_kv = pp1.tile([1, SL], f32, name="ssq_kv")
                kv_sb = []
                for o in range(KCH):
                    ps = a_chunk(o, QLR + 128 * o, 128, "kv")
                    sb = rp.tile([128, SL], f32, name=f"kv_sb{o}")
                    nc.vector.tensor_copy(sb[:], ps[:])
                    kv_sb.append(sb)
                    sq = tp.tile([128, SL], dt, name=f"sqk{o}", tag="sq", bufs=2)
                    nc.scalar.activation(sq[:], ps[:], AF.Square)
                    nc.tensor.matmul(ssq_kv[:], ocol[:], sq[:], start=(o == 0), stop=(o == KCH - 1))
                ps_pe = a_chunk(0, QLR + KVLR, ROPE, "pe")

                # k_pe rope (transposed layout, exact f32 tables)
                tb = pp1.tile([ROPE, SL], f32, name="tb_pe")
                nc.tensor.matmul(tb[:], orow[0:1, 0:ROPE], pos_t[:], start=True, stop=True)
                emb = tp.tile([ROPE, SL], f32, name="emb_pe", tag="emb", bufs=1)
                nc.vector.tensor_scalar(out=emb[:], in0=tb[:], scalar1=invc_t[:], scalar2=None, op0=ALU.mult)
                sin_t = _range_reduce_sin(nc, tp, emb[:], ROPE, SL, 0.0, "sa_s", "sin_s")
                cos_t = _range_reduce_sin(nc, tp, emb[:], ROPE, SL, PI / 2, "sa_c", "sin_c")
                krot = tp.tile([ROPE, SL], f32, name="krot", tag="krot", bufs=1)
                nc.vector.tensor_scalar(out=krot[0:32, :], in0=ps_pe[32:64, :], scalar1=-1.0, scalar2=None, op0=ALU.mult)
                nc.vector.tensor_copy(krot[32:64, :], ps_pe[0:32, :])
                kro = tp.tile([ROPE, SL], f32, name="kro", tag="kro", bufs=1)
                nc.vector.tensor_mul(kro[:], ps_pe[:], cos_t[:])
                krs = tp.tile([ROPE, SL], f32, name="krs", tag="krs", bufs=1)
                nc.vector.tensor_mul(krs[:], krot[:], sin_t[:])
                kfin = tp.tile([ROPE, SL], dt, name="kfin", tag="kfin", bufs=1)
                nc.vector.tensor_add(kfin[:], kro[:], krs[:])
                nc.scalar.dma_start(out=ag_in_kv[KCH * 128:KCH * 128 + ROPE, :], in_=kfin[:])

                # kv rms scale + store
                ms_kv = tp.tile([1, SL], f32, name="ms_kv", tag="ms", bufs=2)
                nc.scalar.activation(ms_kv[:], ssq_kv[:], AF.Sqrt, scale=1.0 / KVLR)
                rkv = tp.tile([1, SL], f32r, name="rkv", tag="rr", bufs=2)
                with nc.allow_low_precision(reason="f32r rounding of rms scale"):
                    nc.vector.reciprocal(rkv[:], ms_kv[:])
                bc_kv = pp1.tile([128, SL], f32, name="bc_kv")
                nc.tensor.matmul(bc_kv[:], orow[:], rkv[:], start=True, stop=True)
                for o in range(KCH):
                    sc = tp.tile([128, SL], dt, name=f"sck{o}", tag="sc", bufs=3)
                    nc.vector.tensor_mul(sc[:], kv_sb[o][:], bc_kv[:])
                    nc.scalar.dma_start(out=ag_in_kv[128 * o:128 * (o + 1), :], in_=sc[:])

                nc.gpsimd.collective_compute(
                    "AllGather", ALU.bypass,
                    replica_groups=[list(range(NC_))],
                    ins=[ag_in_kv[:]], outs=[ag_out_kv[:]],
                )

                # q chunks
                ssq_q = pp1.tile([1, SL], f32, name="ssq_q")
                qa_sb = []
                for o in range(QCH):
                    ps = a_chunk(o, 128 * o, 128, "q")
                    sb = rp.tile([128, SL], f32, name=f"qa_sb{o}")
                    nc.vector.tensor_copy(sb[:], ps[:])
                    qa_sb.append(sb)
                    sq = tp.tile([128, SL], dt, name=f"sqq{o}", tag="sq", bufs=2)
                    nc.scalar.activation(sq[:], ps[:], AF.Square)
                    nc.tensor.matmul(ssq_q[:], ocol[:], sq[:], start=(o == 0), stop=(o == QCH - 1))
                ms_q = tp.tile([1, SL], f32, name="ms_q", tag="ms", bufs=2)
                nc.scalar.activation(ms_q[:], ssq_q[:], AF.Sqrt, scale=1.0 / QLR)
                rq = tp.tile([1, SL], f32r, name="rq", tag="rr", bufs=2)
                with nc.allow_low_precision(reason="f32r rounding of rms scale"):
                    nc.vector.reciprocal(rq[:], ms_q[:])
                bc_q = pp1.tile([128, SL], f32, name="bc_q")
                nc.tensor.matmul(bc_q[:], orow[:], rq[:], start=True, stop=True)
                for o in range(QCH):
                    sc = tp.tile([128, SL], dt, name=f"scq{o}", tag="sc", bufs=3)
                    nc.vector.tensor_mul(sc[:], qa_sb[o][:], bc_q[:])
                    nc.scalar.dma_start(out=ag_in_q[128 * o:128 * (o + 1), :], in_=sc[:])

                nc.gpsimd.collective_compute(
                    "AllGather", ALU.bypass,
                    replica_groups=[list(range(NC_))],
                    ins=[ag_in_q[:]], outs=[ag_out_q[:]],
                )

            agkv_r = ag_out_kv.rearrange("(r c) q -> r c q", r=NC_)
            agq_r = ag_out_q.rearrange("(r c) q -> r c q", r=NC_)

            # ================= Stage B: head-parallel attention =================
            with tc.tile_pool(name="sb_res", bufs=1) as rp, \
                 tc.tile_pool(name="sb_qa", bufs=2) as qap, \
                 tc.tile_pool(name="sb_tmp", bufs=2) as tp, \
                 tc.tile_pool(name="sb_pt", bufs=4) as ptp, \
                 tc.tile_pool(name="sb_wo", bufs=1) as wsp, \
                 tc.tile_pool(name="sb_ag", bufs=1) as agp, \
                 tc.tile_pool(name="sb_ps", bufs=2, space="PSUM") as pp, \
                 tc.tile_pool(name="sb_ps1", bufs=1, space="PSUM") as pp1:
                for col in range(HID // 512):
                    for c in range(HCH):
                        t = wsp.tile([128, 512], dt, name=f"wo_s{c}_{col}", tag="wo_s", bufs=40)
                        nc.sync.dma_start(out=t[:], in_=wo[128 * c:128 * (c + 1), 512 * col:512 * (col + 1)])
                        wo_map[(c, col)] = t[:]
                att_g = {}

                kpe_g = rp.tile([ROPE, S], dt, name="kpe_g")
                for r in range(NC_):
                    nc.sync.dma_start(out=kpe_g[:, SL * r:SL * (r + 1)],
                                      in_=agkv_r[r, KCH * 128:KCH * 128 + ROPE, :])

                wqb_t = []
                for l in range(QCH):
                    t = rp.tile([128, HPC * 256], dt, name=f"wqb_t{l}")
                    nc.sync.dma_start(out=t[:], in_=wqb[128 * l:128 * (l + 1), :])
                    wqb_t.append(t)
                wkk_t = []
                wkv_t = []
                for l in range(KCH):
                    t = rp.tile([128, HPC * NOPE], dt, name=f"wkk_t{l}")
                    nc.sync.dma_start(out=t[:], in_=wkvb_k[128 * l:128 * (l + 1), :])
                    wkk_t.append(t)
                    t2 = rp.tile([128, HPC * VD], dt, name=f"wkv_t{l}")
                    nc.sync.dma_start(out=t2[:], in_=wkvb_v[128 * l:128 * (l + 1), :])
                    wkv_t.append(t2)

                # K^T and V (both heads); ckv_g freed afterwards
                kT = [rp.tile([128, S], dt, name=f"kT{h}") for h in range(HPC)]
                v_t = [rp.tile([128, HPC * VD], dt, name=f"v_t{kb}") for kb in range(NKB)]
                with tc.tile_pool(name="sb_ckv", bufs=1) as ckvp:
                    ckv_g = []
                    for j in range(KCH):
                        t = ckvp.tile([128, S], dt, name=f"ckv_g{j}")
                        for r in range(NC_):
                            nc.sync.dma_start(out=t[:, SL * r:SL * (r + 1)],
                                              in_=agkv_r[r, 128 * j:128 * (j + 1), :])
                        ckv_g.append(t)
                    for h in range(HPC):
                        for kc in range(S // 512):
                            ps = pp.tile([128, 512], f32, name=f"kt_ps{h}_{kc}", tag="mm_ps", bufs=2)
                            for l in range(KCH):
                                nc.tensor.matmul(ps[:], wkk_t[l][:, NOPE * h:NOPE * (h + 1)],
                                                 ckv_g[l][:, 512 * kc:512 * (kc + 1)],
                                                 start=(l == 0), stop=(l == KCH - 1))
                            nc.vector.tensor_copy(kT[h][:, 512 * kc:512 * (kc + 1)], ps[:])
                    for kb in range(NKB):
                        ps = pp.tile([128, HPC * VD], f32, name=f"v_ps{kb}", tag="mm_ps", bufs=2)
                        for l in range(KCH):
                            nc.tensor.matmul(ps[:], ckv_g[l][:, 128 * kb:128 * (kb + 1)], wkv_t[l][:],
                                             start=(l == 0), stop=(l == KCH - 1))
                        nc.vector.tensor_copy(v_t[kb][:], ps[:])

                # ---- precompute all q projections (qn + roped qp), panel-major ----
                qn_sb = {}
                qp_sb = {}
                for p in range(NPANEL):
                    qs = slice(PANEL * p, PANEL * (p + 1))
                    qa_p = []
                    for l in range(QCH):
                        t = qap.tile([128, PANEL], dt, name=f"qa_p{p}_{l}", tag=f"qa_p{l}", bufs=2)
                        for r in range(2):
                            nc.sync.dma_start(out=t[:, SL * r:SL * (r + 1)],
                                              in_=agq_r[2 * p + r, 128 * l:128 * (l + 1), :])
                        qa_p.append(t)
                    tb = pp1.tile([ROPE, PANEL], f32, name=f"tbp{p}", tag="bc_ps", bufs=1)
                    nc.tensor.matmul(tb[:], orow[0:1, 0:ROPE], pos_all_t[:, qs], start=True, stop=True)
                    embp = tp.tile([ROPE, PANEL], f32, name=f"embp{p}", tag="embp", bufs=2)
                    nc.vector.tensor_scalar(out=embp[:], in0=tb[:], scalar1=invc_t[:], scalar2=None, op0=ALU.mult)
                    sin_p = _range_reduce_sin(nc, tp, embp[:], ROPE, PANEL, 0.0, f"sb_s{p}", "sin_s")
                    cos_p = _range_reduce_sin(nc, tp, embp[:], ROPE, PANEL, PI / 2, f"sb_c{p}", "sin_c")
                    for h in range(HPC):
                        hcol = 256 * h
                        ps_qn = pp.tile([128, PANEL], f32, name=f"qn_ps{h}_{p}", tag="mm_ps", bufs=2)
                        for l in range(QCH):
                            nc.tensor.matmul(ps_qn[:], wqb_t[l][:, hcol:hcol + NOPE], qa_p[l][:],
                                             start=(l == 0), stop=(l == QCH - 1))
                        ps_qr = pp.tile([128, PANEL], f32, name=f"qr_ps{h}_{p}", tag="mm_ps", bufs=2)
                        for l in range(QCH):
                            nc.tensor.matmul(ps_qr[:], wqb_t[l][:, hcol + NOPE:hcol + 256], qa_p[l][:],
                                             start=(l == 0), stop=(l == QCH - 1))
                        qn = rp.tile([128, PANEL], dt, name=f"qn_sb{h}_{p}")
                        nc.vector.tensor_copy(qn[:], ps_qn[:])
                        qn_sb[(h, p)] = qn
                        qt1 = tp.tile([ROPE, PANEL], f32, name=f"qt1_{h}_{p}", tag="qt1", bufs=2)
                        nc.vector.tensor_mul(qt1[:], ps_qr[0:ROPE, :], cos_p[:])
                        qt2 = tp.tile([ROPE, PANEL], f32, name=f"qt2_{h}_{p}", tag="qt2", bufs=2)
                        nc.vector.tensor_mul(qt2[:], ps_qr[ROPE:2 * ROPE, :], sin_p[:])
                        qp = rp.tile([ROPE, PANEL], dt, name=f"qp_sb{h}_{p}")
                        nc.vector.tensor_add(qp[:], qt1[:], qt2[:])
                        qp_sb[(h, p)] = qp

                # ---- attention ----
                for h in range(HPC):
                    for p in range(NPANEL):
                        nkb = 4 * (p + 1)
                        ps_at = pp.tile([128, PANEL], f32, name=f"at_ps{h}_{p}", tag="at_ps", bufs=1)
                        ps_sum = pp1.tile([1, PANEL], f32, name=f"sum_ps{h}_{p}", tag="sum_ps", bufs=1)
                        pts = {}

                        def consume(kb):
                            nc.tensor.matmul(ps_sum[:], ocol[:], pts[kb][:],
                                             start=(kb == 0), stop=(kb == nkb - 1))
                            nc.tensor.matmul(ps_at[:], v_t[kb][:, VD * h:VD * (h + 1)], pts[kb][:],
                                             start=(kb == 0), stop=(kb == nkb - 1))

                        for kb in range(nkb):
                            ps_sc = pp.tile([128, PANEL], f32, name=f"sc_ps{h}_{p}_{kb}", tag="sc_ps", bufs=3)
                            nc.tensor.matmul(ps_sc[:], kT[h][:, 128 * kb:128 * (kb + 1)], qn_sb[(h, p)][:],
                                             start=True, stop=False)
                            nc.tensor.matmul(ps_sc[:], kpe_g[:, 128 * kb:128 * (kb + 1)], qp_sb[(h, p)][:],
                                             start=False, stop=True)
                            pt = ptp.tile([128, PANEL], dt, name=f"pt{h}_{p}_{kb}", tag="pt", bufs=4)
                            nc.scalar.activation(pt[:], ps_sc[:], AF.Exp, scale=SM_SCALE)
                            if kb >= 4 * p:
                                j = kb - 4 * p
                                nc.gpsimd.affine_select(
                                    out=pt[:], in_=pt[:],
                                    pattern=[[1, PANEL]],
                                    compare_op=ALU.is_ge,
                                    fill=0.0,
                                    base=-128 * j,
                                    channel_multiplier=-1)
                            pts[kb] = pt
                            if kb > 0:
                                consume(kb - 1)
                        consume(nkb - 1)
                        rec = tp.tile([1, PANEL], f32r, name=f"rec{h}_{p}", tag="rec", bufs=2)
                        with nc.allow_low_precision(reason="f32r rounding of softmax recip"):
                            nc.vector.reciprocal(rec[:], ps_sum[:])
                        bc = pp1.tile([128, PANEL], f32, name=f"bc_ps{h}_{p}", tag="bc_ps", bufs=1)
                        nc.tensor.matmul(bc[:], orow[:], rec[:], start=True, stop=True)
                        bc_sb = tp.tile([128, PANEL], f32, name=f"bc_sb{h}_{p}", tag="bc_sb", bufs=2)
                        nc.vector.tensor_copy(bc_sb[:], bc[:])
                        at_p = tp.tile([128, PANEL], dt, name=f"at_p{h}_{p}", tag="at_p", bufs=2)
                        nc.vector.tensor_mul(at_p[:], ps_at[:], bc_sb[:])
                        for r in range(2):
                            j = 2 * p + r
                            nc.scalar.dma_start(
                                out=a2a_in[h][j * VD:(j + 1) * VD, :],
                                in_=at_p[:, SL * r:SL * (r + 1)])
                    nc.gpsimd.collective_compute(
                        "AllToAll", ALU.bypass,
                        replica_groups=[list(range(NC_))],
                        ins=[a2a_in[h][:]], outs=[a2a_out[h][:]],
                    )
                    for j in range(NC_):
                        c = 2 * j + h
                        t = agp.tile([128, SL], dt, name=f"att_g{c}")
                        nc.sync.dma_start(out=t[:], in_=a2a_out[h][128 * j:128 * (j + 1), :])
                        att_g[c] = t

                # ---- Wo: seq-parallel output projection ----
                for col in range(HID // 512):
                    for qb in range(SL // 128):
                        ps = pp.tile([128, 512], f32, name=f"o_ps{col}_{qb}", tag="mm_ps", bufs=2)
                        for c in range(HCH):
                            nc.tensor.matmul(ps[:], att_g[c][:, 128 * qb:128 * (qb + 1)], wo_map[(c, col)],
                                             start=(c == 0), stop=(c == HCH - 1))
                        osb = tp.tile([128, 512], f32, name=f"osb{col}_{qb}", tag="osb", bufs=3)
                        nc.vector.tensor_copy(osb[:], ps[:])
                        nc.sync.dma_start(out=out_loc[128 * qb:128 * (qb + 1), 512 * col:512 * (col + 1)], in_=osb[:])

    nc.compile()
    return nc


def _to_dt(a, dt):
    if dt == bf16:
        return np.ascontiguousarray(a.astype(ml_dtypes.bfloat16))
    return np.ascontiguousarray(a.astype(np.float32))


def _prepare_inputs(dt, hidden_states, position_ids, Wqa, qa_ln_w, Wqb, Wkva, kv_ln_w, Wkvb, Wo):
    perm = np.concatenate([np.arange(0, ROPE, 2), np.arange(1, ROPE, 2)])
    X = np.asarray(hidden_states, np.float32).reshape(S, HID)
    pos_f = np.ascontiguousarray(np.asarray(position_ids, np.float32).reshape(1, S))
    Wqa = np.asarray(Wqa, np.float32)
    Wkva = np.asarray(Wkva, np.float32)
    wa = np.concatenate([Wqa, Wkva[:, :KVLR], Wkva[:, KVLR:][:, perm]], axis=1)  # (2048, 2112)
    wqb_base = np.asarray(Wqb, np.float32) * np.asarray(qa_ln_w, np.float32)[:, None]
    wkvb_base = np.asarray(Wkvb, np.float32) * np.asarray(kv_ln_w, np.float32)[:, None]
    Wo = np.asarray(Wo, np.float32)

    head_blocks = []
    for h in range(NH):
        cols = wqb_base[:, 192 * h:192 * (h + 1)]
        nope = cols[:, :NOPE]
        pe_d = cols[:, NOPE:][:, perm]
        rot = np.concatenate([-pe_d[:, 32:], pe_d[:, :32]], axis=1)
        head_blocks.append(np.concatenate([nope, pe_d, rot], axis=1))  # (1536, 256)
    k_blocks = [wkvb_base[:, 256 * h:256 * h + NOPE] for h in range(NH)]
    v_blocks = [wkvb_base[:, 256 * h + NOPE:256 * (h + 1)] for h in range(NH)]

    inv = (1.0 / (THETA ** (np.arange(0, ROPE, 2, dtype=np.float32) / ROPE))).astype(np.float32)
    invf_np = np.concatenate([inv, inv])

    wa_d = _to_dt(wa, dt)
    wo_d = _to_dt(Wo, dt)
    ones_col_d = _to_dt(np.ones((128, 1), np.float32), dt)

    in_maps = []
    for c in range(NC_):
        rows = slice(SL * c, SL * (c + 1))
        in_maps.append({
            "x_t": _to_dt(X[rows, :].T, dt),
            "pos": np.ascontiguousarray(pos_f[:, rows]),
            "pos_all": pos_f,
            "wa": wa_d,
            "wqb": _to_dt(np.concatenate([head_blocks[HPC * c + h] for h in range(HPC)], axis=1), dt),
            "wkvb_k": _to_dt(np.concatenate([k_blocks[HPC * c + h] for h in range(HPC)], axis=1), dt),
            "wkvb_v": _to_dt(np.concatenate([v_blocks[HPC * c + h] for h in range(HPC)], axis=1), dt),
            "wo": wo_d,
            "ones_col": ones_col_d,
            "ones_row": np.ones((1, 128), np.float32),
            "invf_col": invf_np.reshape(ROPE, 1).copy(),
        })
    return in_maps


def run(inputs, trace=False, trace_cores=None, dt=None):
    dt = dt if dt is not None else DT
    key = ("nc", str(dt))
    if key not in _CACHE:
        _CACHE[key] = build_program(dt)
    nc = _CACHE[key]
    in_maps = _prepare_inputs(dt, **inputs)
    res = run_bass_kernel_spmd(nc, in_maps, list(range(NC_)), trace=trace,
                               trace_cores=trace_cores)
    out = np.concatenate([res.results[c]["out_loc"] for c in range(NC_)], axis=0)
    return out.reshape(1, S, HID), res


def kernel(**inputs) -> np.ndarray:
    out, _ = run(inputs, trace=False)
    return out



# revision 10
# speedup vs baseline: 1.1305x; 1.1305x over previous
"""DeepseekV3 MLA flash-attention prefill kernel for 8 Trainium2 NeuronCores.

Sharding (SPMD, one program for all 8 cores):
  Stage A (sequence-parallel): core c computes low-rank down-projections
    q_a = rms(X @ Wqa), c_kv = rms(ckv[:, :512]), roped k_pe for its 256
    rows in transposed layout, AllGathers kv first (early trigger) then q.
  Stage B (head-parallel): core c owns heads {2c, 2c+1}: K^T/V built while
    the q AllGather is in flight; q projection and causal attention are
    interleaved per 512-wide panel; softmax runs in (k, q) layout without
    max subtraction; per-q normalization is folded into the attn^T
    eviction with the reciprocal double-buffered off the critical path.
  Output: per-head AllToAll; Wo runs in two halves (h0 half during the
    h1 AllToAll flight, accumulating in parked PSUM banks).

Perf notes vs v1: rope sin/cos tables come from the host (one ACT table
set: ln/exp/square, no swaps), both AllGathers trigger ~60us earlier, a
tiny warm-up collective absorbs first-collective latency, keep-warm
matmul chains hold the PE HAM clock at 2.4GHz across DMA/collective
gaps, diagonal score blocks skip fully-masked columns.
"""

import sys

if '/opt/trn_rl_repo' not in sys.path:
    sys.path.insert(0, '/opt/trn_rl_repo')

import numpy as np
import ml_dtypes

import concourse.bass as bass
import concourse.mybir as mybir
import concourse.tile as tile
from concourse import bacc
from concourse.bass_utils import run_bass_kernel_spmd

f32 = mybir.dt.float32
f32r = mybir.dt.float32r
bf16 = mybir.dt.bfloat16
i32 = mybir.dt.int32
AF = mybir.ActivationFunctionType
ALU = mybir.AluOpType

NC_ = 8            # cores
S = 2048           # sequence
HID = 2048
QLR = 1536         # q lora rank
KVLR = 512         # kv lora rank
ROPE = 64
NOPE = 128
VD = 128
NH = 16
HPC = NH // NC_    # heads per core = 2
SL = S // NC_      # rows per core = 256
PANEL = 512        # q panel width
NPANEL = S // PANEL
NKB = S // 128     # 16 k blocks
QCH = QLR // 128   # 12
KCH = KVLR // 128  # 4
HCH = HID // 128   # 16
THETA = 10000.0
SM_SCALE = float((NOPE + ROPE) ** -0.5)

DT = bf16          # matmul dtype

NWG = 8            # wa_q DMA sub-groups (trickle arrival keeps HAM warm)
WARM1 = 10         # keep-warm links during AG-q wait

_CACHE = {}


def build_program(dt):
    nc = bacc.Bacc("TRN2", target_bir_lowering=False, debug=False, num_devices=NC_)

    def din(name, shape, d=None):
        return nc.dram_tensor(name, shape, d or dt, kind="ExternalInput")

    # ---- external I/O ----
    # x_t packed hc-major: 4 groups of [128, 4*256]
    x_t = din("x_t", [128, HCH * SL])
    # wa_kv packed: per hc 576 cols ([kv 512 | pe 64]); 4 groups of 4 hc
    wa_kv = din("wa_kv", [128, HCH * (KVLR + ROPE)])
    # wa_q packed: per hc 1536 cols; NWG groups of HCH//NWG hc
    wa_q = din("wa_q", [128, HCH * QLR])
    # wqb packed: per l in 0..11, 512 cols ([nope|pe_d|rot] x 2 heads)
    wqb = din("wqb", [128, QCH * HPC * 256])
    wkvb_k = din("wkvb_k", [128, KCH * HPC * NOPE])
    wkvb_v = din("wkvb_v", [128, KCH * HPC * VD])
    wo = din("wo", [NH * VD, HID])
    ones_col = din("ones_col", [128, 1])
    ones_row = nc.dram_tensor("ones_row", [1, 128], f32, kind="ExternalInput")
    sin_all = nc.dram_tensor("sin_all", [ROPE, S], f32, kind="ExternalInput")
    cos_all = nc.dram_tensor("cos_all", [ROPE, S], f32, kind="ExternalInput")
    sin_loc = nc.dram_tensor("sin_loc", [ROPE, SL], f32, kind="ExternalInput")
    cos_loc = nc.dram_tensor("cos_loc", [ROPE, SL], f32, kind="ExternalInput")
    out_loc = nc.dram_tensor("out_loc", [SL, HID], f32, kind="ExternalOutput")

    NAG_KV = KCH + 1
    HG = HCH // 4          # hc per wa_kv/x group = 4
    QHG = HCH // NWG       # hc per wa_q group = 2

    with tile.TileContext(nc) as tc:
        with tc.tile_pool(name="dram", bufs=1, space="DRAM") as dpool, \
             tc.tile_pool(name="consts", bufs=1) as cpool:
            agw_in = dpool.tile([128, 1], dt)
            agw_out = dpool.tile([NC_ * 128, 1], dt, addr_space="Shared")
            ag_in_kv = dpool.tile([NAG_KV * 128, SL], dt)
            ag_out_kv = dpool.tile([NC_ * NAG_KV * 128, SL], dt, addr_space="Shared")
            ag_in_q = dpool.tile([QCH * 128, SL], dt)
            ag_out_q = dpool.tile([NC_ * QCH * 128, SL], dt, addr_space="Shared")
            a2a_in = [dpool.tile([NC_ * VD, SL], dt, name=f"a2a_in{h}") for h in range(HPC)]
            a2a_out = [dpool.tile([NC_ * VD, SL], dt, name=f"a2a_out{h}") for h in range(HPC)]

            ocol = cpool.tile([128, 1], dt)
            orow = cpool.tile([1, 128], f32r)
            sin_g = cpool.tile([ROPE, S], f32)
            cos_g = cpool.tile([ROPE, S], f32)
            sin_l = cpool.tile([ROPE, SL], f32)
            cos_l = cpool.tile([ROPE, SL], f32)
            nc.sync.dma_start(out=ocol[:], in_=ones_col[:])
            nc.sync.dma_start(out=orow[:], in_=ones_row[:].bitcast(f32r))
            nc.scalar.dma_start(out=sin_l[:], in_=sin_loc[:])
            nc.scalar.dma_start(out=cos_l[:], in_=cos_loc[:])
            nc.scalar.dma_start(out=sin_g[:], in_=sin_all[:])
            nc.scalar.dma_start(out=cos_g[:], in_=cos_all[:])
            # warm-up collective: absorbs first-collective latency early
            nc.scalar.dma_start(out=agw_in[:], in_=ones_col[:])
            nc.gpsimd.collective_compute(
                "AllGather", ALU.bypass,
                replica_groups=[list(range(NC_))],
                ins=[agw_in[:]], outs=[agw_out[:]],
            )

            # ================= Stage A: transposed down projections =================
            with tc.tile_pool(name="sa_x", bufs=1) as xp, \
                 tc.tile_pool(name="sa_w", bufs=1) as wp, \
                 tc.tile_pool(name="sa_res", bufs=1) as rp_a, \
                 tc.tile_pool(name="sa_tmp", bufs=2) as tp, \
                 tc.tile_pool(name="sa_ps", bufs=2, space="PSUM") as pp, \
                 tc.tile_pool(name="sa_ps1", bufs=1, space="PSUM") as pp1:

                x_g = []
                wakv_g = []
                for g in range(4):
                    xt = xp.tile([128, HG * SL], dt, name=f"xg{g}")
                    nc.sync.dma_start(out=xt[:], in_=x_t[:, HG * SL * g:HG * SL * (g + 1)])
                    x_g.append(xt)
                    wt = wp.tile([128, HG * 576], dt, name=f"wakv{g}")
                    nc.sync.dma_start(out=wt[:], in_=wa_kv[:, HG * 576 * g:HG * 576 * (g + 1)])
                    wakv_g.append(wt)
                waq_g = []
                for g in range(NWG):
                    wt = wp.tile([128, QHG * QLR], dt, name=f"waq{g}")
                    nc.sync.dma_start(out=wt[:], in_=wa_q[:, QHG * QLR * g:QHG * QLR * (g + 1)])
                    waq_g.append(wt)

                def xs(hc):
                    return x_g[hc // HG][:, SL * (hc % HG):SL * (hc % HG + 1)]

                # kv chunks + ssq (ssq matmul for chunk o-1 emitted under
                # chunk o's matmuls so the PE never waits on the square)
                ssq_kv = pp1.tile([1, SL], f32, name="ssq_kv")
                kv_sb = []
                prev_sq = None
                for o in range(KCH):
                    ps = pp.tile([128, SL], f32, name=f"ps_kv{o}", tag="a_ps", bufs=2)
                    for hc in range(HCH):
                        w = wakv_g[hc // HG]
                        c0 = 576 * (hc % HG) + 128 * o
                        nc.tensor.matmul(ps[:], w[:, c0:c0 + 128], xs(hc),
                                         start=(hc == 0), stop=(hc == HCH - 1))
                    if prev_sq is not None:
                        nc.tensor.matmul(ssq_kv[:], ocol[:], prev_sq[:], start=(o == 1), stop=False)
                    sb = rp_a.tile([128, SL], f32, name=f"kv_sb{o}")
                    nc.vector.tensor_copy(sb[:], ps[:])
                    kv_sb.append(sb)
                    sq = tp.tile([128, SL], dt, name=f"sqk{o}", tag="sq", bufs=2)
                    nc.scalar.activation(sq[:], ps[:], AF.Square)
                    prev_sq = sq

                # pe chunk + rope (host sin/cos slices)
                ps_pe = pp.tile([ROPE, SL], f32, name="ps_pe", tag="a_ps", bufs=2)
                for hc in range(HCH):
                    w = wakv_g[hc // HG]
                    c0 = 576 * (hc % HG) + KVLR
                    nc.tensor.matmul(ps_pe[:], w[:, c0:c0 + ROPE], xs(hc),
                                     start=(hc == 0), stop=(hc == HCH - 1))
                nc.tensor.matmul(ssq_kv[:], ocol[:], prev_sq[:], start=False, stop=True)
                krot = tp.tile([ROPE, SL], f32, name="krot", tag="krot", bufs=1)
                nc.vector.tensor_scalar(out=krot[0:32, :], in0=ps_pe[32:64, :], scalar1=-1.0, scalar2=None, op0=ALU.mult)
                nc.vector.tensor_copy(krot[32:64, :], ps_pe[0:32, :])
                kro = tp.tile([ROPE, SL], f32, name="kro", tag="kro", bufs=1)
                nc.vector.tensor_mul(kro[:], ps_pe[:], cos_l[:])
                krs = tp.tile([ROPE, SL], f32, name="krs", tag="krs", bufs=1)
                nc.vector.tensor_mul(krs[:], krot[:], sin_l[:])
                kfin = tp.tile([ROPE, SL], dt, name="kfin", tag="kfin", bufs=1)
                nc.vector.tensor_add(kfin[:], kro[:], krs[:])
                nc.scalar.dma_start(out=ag_in_kv[KCH * 128:KCH * 128 + ROPE, :], in_=kfin[:])

                # kv rms scale via exp(-0.5*ln(ssq/KVLR)) on scalar engine
                ln_kv = tp.tile([1, SL], f32, name="ln_kv", tag="lnr", bufs=2)
                nc.scalar.activation(ln_kv[:], ssq_kv[:], AF.Ln, scale=1.0 / KVLR)
                rkv = tp.tile([1, SL], f32r, name="rkv", tag="rr", bufs=2)
                with nc.allow_low_precision(reason="f32r rounding of rms scale"):
                    nc.scalar.activation(rkv[:], ln_kv[:], AF.Exp, scale=-0.5)
                bc_kv = pp1.tile([128, SL], f32, name="bc_kv")
                nc.tensor.matmul(bc_kv[:], orow[:], rkv[:], start=True, stop=True)
                for o in range(KCH):
                    sc = tp.tile([128, SL], dt, name=f"sck{o}", tag="sc", bufs=3)
                    nc.vector.tensor_mul(sc[:], kv_sb[o][:], bc_kv[:])
                    nc.scalar.dma_start(out=ag_in_kv[128 * o:128 * (o + 1), :], in_=sc[:])

                nc.gpsimd.collective_compute(
                    "AllGather", ALU.bypass,
                    replica_groups=[list(range(NC_))],
                    ins=[ag_in_kv[:]], outs=[ag_out_kv[:]],
                )

                # q chunks (wa_q trickles in; emit g-ascending per chunk)
                ssq_q = pp1.tile([1, SL], f32, name="ssq_q")
                qa_sb = []
                prev_sq = None
                for o in range(QCH):
                    ps = pp.tile([128, SL], f32, name=f"ps_q{o}", tag="a_ps", bufs=2)
                    for hc in range(HCH):
                        w = waq_g[hc // QHG]
                        c0 = QLR * (hc % QHG) + 128 * o
                        nc.tensor.matmul(ps[:], w[:, c0:c0 + 128], xs(hc),
                                         start=(hc == 0), stop=(hc == HCH - 1))
                    if prev_sq is not None:
                        nc.tensor.matmul(ssq_q[:], ocol[:], prev_sq[:], start=(o == 1), stop=False)
                    sb = rp_a.tile([128, SL], f32, name=f"qa_sb{o}")
                    nc.vector.tensor_copy(sb[:], ps[:])
                    qa_sb.append(sb)
                    sq = tp.tile([128, SL], dt, name=f"sqq{o}", tag="sq", bufs=2)
                    nc.scalar.activation(sq[:], ps[:], AF.Square)
                    prev_sq = sq
                nc.tensor.matmul(ssq_q[:], ocol[:], prev_sq[:], start=False, stop=True)
                ln_q = tp.tile([1, SL], f32, name="ln_q", tag="lnr", bufs=2)
                nc.scalar.activation(ln_q[:], ssq_q[:], AF.Ln, scale=1.0 / QLR)
                rq = tp.tile([1, SL], f32r, name="rq", tag="rr", bufs=2)
                with nc.allow_low_precision(reason="f32r rounding of rms scale"):
                    nc.scalar.activation(rq[:], ln_q[:], AF.Exp, scale=-0.5)
                bc_q = pp1.tile([128, SL], f32, name="bc_q")
                nc.tensor.matmul(bc_q[:], orow[:], rq[:], start=True, stop=True)
                for o in range(QCH):
                    sc = tp.tile([128, SL], dt, name=f"scq{o}", tag="sc", bufs=3)
                    nc.vector.tensor_mul(sc[:], qa_sb[o][:], bc_q[:])
                    nc.scalar.dma_start(out=ag_in_q[128 * o:128 * (o + 1), :], in_=sc[:])

                nc.gpsimd.collective_compute(
                    "AllGather", ALU.bypass,
                    replica_groups=[list(range(NC_))],
                    ins=[ag_in_q[:]], outs=[ag_out_q[:]],
                )

            agkv_r = ag_out_kv.rearrange("(r c) q -> r c q", r=NC_)
            agq_r = ag_out_q.rearrange("(r c) q -> r c q", r=NC_)

            # ================= Stage B: head-parallel attention =================
            with tc.tile_pool(name="sb_res", bufs=1) as rp, \
                 tc.tile_pool(name="sb_wo", bufs=1) as wsp, \
                 tc.tile_pool(name="sb_ag", bufs=1) as agp:
                att_g = {}
                wo_t = []
                qn_sb = {}
                qp_sb = {}

                with tc.tile_pool(name="sb_qa", bufs=2) as qap, \
                     tc.tile_pool(name="sb_tmp", bufs=2) as tp, \
                     tc.tile_pool(name="sb_pt", bufs=6) as ptp, \
                     tc.tile_pool(name="sb_ps", bufs=3, space="PSUM") as pp, \
                     tc.tile_pool(name="sb_psat", bufs=2, space="PSUM") as ppat, \
                     tc.tile_pool(name="sb_ps1", bufs=1, space="PSUM") as pp1:

                    # weights for K/V and q projections (scalar queue)
                    wkk_t = rp.tile([128, KCH * HPC * NOPE], dt, name="wkk_t")
                    nc.scalar.dma_start(out=wkk_t[:], in_=wkvb_k[:])
                    wkv_t = rp.tile([128, KCH * HPC * VD], dt, name="wkv_t")
                    nc.scalar.dma_start(out=wkv_t[:], in_=wkvb_v[:])
                    wqb_t = rp.tile([128, QCH * HPC * 256], dt, name="wqb_t")
                    nc.scalar.dma_start(out=wqb_t[:], in_=wqb[:])

                    # gathered kv (sync queue)
                    kpe_g = rp.tile([ROPE, S], dt, name="kpe_g")
                    for r in range(NC_):
                        nc.sync.dma_start(out=kpe_g[:, SL * r:SL * (r + 1)],
                                          in_=agkv_r[r, KCH * 128:KCH * 128 + ROPE, :])
                    kT = [rp.tile([128, S], dt, name=f"kT{h}") for h in range(HPC)]
                    v_t = [rp.tile([128, HPC * VD], dt, name=f"v_t{kb}") for kb in range(NKB)]
                    with tc.tile_pool(name="sb_ckv", bufs=1) as ckvp:
                        ckv_g = []
                        for j in range(KCH):
                            t = ckvp.tile([128, S], dt, name=f"ckv_g{j}")
                            for r in range(NC_):
                                nc.sync.dma_start(out=t[:, SL * r:SL * (r + 1)],
                                                  in_=agkv_r[r, 128 * j:128 * (j + 1), :])
                            ckv_g.append(t)
                        for h in range(HPC):
                            for kc in range(S // 512):
                                ps = pp.tile([128, 512], f32, name=f"kt_ps{h}_{kc}", tag="mm_ps", bufs=3)
                                for l in range(KCH):
                                    nc.tensor.matmul(ps[:], wkk_t[:, 256 * l + NOPE * h:256 * l + NOPE * (h + 1)],
                                                     ckv_g[l][:, 512 * kc:512 * (kc + 1)],
                                                     start=(l == 0), stop=(l == KCH - 1))
                                nc.vector.tensor_copy(kT[h][:, 512 * kc:512 * (kc + 1)], ps[:])
                        for kb in range(NKB):
                            ps = pp.tile([128, HPC * VD], f32, name=f"v_ps{kb}", tag="mm_ps", bufs=3)
                            for l in range(KCH):
                                nc.tensor.matmul(ps[:], ckv_g[l][:, 128 * kb:128 * (kb + 1)],
                                                 wkv_t[:, 256 * l:256 * (l + 1)],
                                                 start=(l == 0), stop=(l == KCH - 1))
                            nc.vector.tensor_copy(v_t[kb][:], ps[:])

                    # wo weight loads issued on the vector queue: transfers
                    # run during the proj/attention phase, needed at ~tail
                    for c in range(HCH):
                        t = wsp.tile([128, HID], dt, name=f"wo_t{c}")
                        nc.gpsimd.dma_start(out=t[:], in_=wo[128 * c:128 * (c + 1), :])
                        wo_t.append(t)

                    # keep-warm chain during AG-q flight: tiny serialized
                    # mm->vector links so HAM sees activity every ~1.5us
                    warm_sb = tp.tile([128, 512], dt, name="warm_sb", tag="warm", bufs=1)
                    nc.vector.tensor_copy(warm_sb[:], kT[0][:, 0:512])
                    warm_ps = pp1.tile([1, PANEL], f32, name="warm_ps", tag="sum_ps", bufs=2)
                    wfb = tp.tile([1, 16], f32, name="wfb", tag="wfb", bufs=1)
                    for i in range(WARM1):
                        nc.tensor.matmul(warm_ps[:], ocol[:], warm_sb[:], start=True, stop=True)
                        nc.vector.tensor_copy(wfb[:], warm_ps[:, 0:16])
                        nc.vector.tensor_copy(warm_sb[0:1, 0:16], wfb[:])

                    # ---- per panel: q projection then attention (both heads) ----
                    for p in range(NPANEL):
                        qs = slice(PANEL * p, PANEL * (p + 1))
                        qa_p = []
                        for l in range(QCH):
                            t = qap.tile([128, PANEL], dt, name=f"qa_p{p}_{l}", tag=f"qa_p{l}", bufs=2)
                            for r in range(2):
                                nc.sync.dma_start(out=t[:, SL * r:SL * (r + 1)],
                                                  in_=agq_r[2 * p + r, 128 * l:128 * (l + 1), :])
                            qa_p.append(t)
                        for h in range(HPC):
                            hcol = 256 * h
                            ps_qn = pp.tile([128, PANEL], f32, name=f"qn_ps{h}_{p}", tag="mm_ps", bufs=3)
                            for l in range(QCH):
                                nc.tensor.matmul(ps_qn[:], wqb_t[:, 512 * l + hcol:512 * l + hcol + NOPE],
                                                 qa_p[l][:], start=(l == 0), stop=(l == QCH - 1))
                            ps_qr = pp.tile([128, PANEL], f32, name=f"qr_ps{h}_{p}", tag="mm_ps", bufs=3)
                            for l in range(QCH):
                                nc.tensor.matmul(ps_qr[:], wqb_t[:, 512 * l + hcol + NOPE:512 * l + hcol + 256],
                                                 qa_p[l][:], start=(l == 0), stop=(l == QCH - 1))
                            qn = rp.tile([128, PANEL], dt, name=f"qn_sb{h}_{p}")
                            nc.vector.tensor_copy(qn[:], ps_qn[:])
                            qn_sb[(h, p)] = qn
                            qt1 = tp.tile([ROPE, PANEL], f32, name=f"qt1_{h}_{p}", tag="qt1", bufs=2)
                            nc.vector.tensor_mul(qt1[:], ps_qr[0:ROPE, :], cos_g[:, qs])
                            qt2 = tp.tile([ROPE, PANEL], f32, name=f"qt2_{h}_{p}", tag="qt2", bufs=2)
                            nc.vector.tensor_mul(qt2[:], ps_qr[ROPE:2 * ROPE, :], sin_g[:, qs])
                            qp = rp.tile([ROPE, PANEL], dt, name=f"qp_sb{h}_{p}")
                            nc.vector.tensor_add(qp[:], qt1[:], qt2[:])
                            qp_sb[(h, p)] = qp

                        for h in range(HPC):
                            nkb = 4 * (p + 1)
                            ps_at = ppat.tile([128, PANEL], f32, name=f"at_ps{h}_{p}", tag="at_ps", bufs=2)
                            ps_sum = pp1.tile([1, PANEL], f32, name=f"sum_ps{h}_{p}", tag="sum_ps", bufs=2)
                            pts = {}

                            def consume(kb, nkb=nkb, ps_sum=ps_sum, ps_at=ps_at, pts=pts, h=h):
                                nc.tensor.matmul(ps_sum[:], ocol[:], pts[kb][:],
                                                 start=(kb == 0), stop=(kb == nkb - 1))
                                nc.tensor.matmul(ps_at[:], v_t[kb][:, VD * h:VD * (h + 1)], pts[kb][:],
                                                 start=(kb == 0), stop=(kb == nkb - 1))

                            for kb in range(nkb):
                                j = kb - 4 * p          # >= 0 on diagonal blocks
                                c0 = 128 * j if j > 0 else 0
                                ps_sc = pp.tile([128, PANEL], f32, name=f"sc_ps{h}_{p}_{kb}", tag="mm_ps", bufs=3)
                                nc.tensor.matmul(ps_sc[:, c0:PANEL], kT[h][:, 128 * kb:128 * (kb + 1)],
                                                 qn_sb[(h, p)][:, c0:PANEL], start=True, stop=False)
                                nc.tensor.matmul(ps_sc[:, c0:PANEL], kpe_g[:, 128 * kb:128 * (kb + 1)],
                                                 qp_sb[(h, p)][:, c0:PANEL], start=False, stop=True)
                                pt = ptp.tile([128, PANEL], dt, name=f"pt{h}_{p}_{kb}", tag="pt", bufs=6)
                                nc.scalar.activation(pt[:, c0:PANEL], ps_sc[:, c0:PANEL], AF.Exp, scale=SM_SCALE)
                                if j >= 0:
                                    nc.gpsimd.affine_select(
                                        out=pt[:], in_=pt[:],
                                        pattern=[[1, PANEL]],
                                        compare_op=ALU.is_ge,
                                        fill=0.0,
                                        base=-128 * j,
                                        channel_multiplier=-1)
                                pts[kb] = pt
                                if kb > 0:
                                    consume(kb - 1)
                            consume(nkb - 1)
                            rec = tp.tile([1, PANEL], f32r, name=f"rec{h}_{p}", tag="rec", bufs=2)
                            with nc.allow_low_precision(reason="f32r rounding of softmax recip"):
                                nc.vector.reciprocal(rec[:], ps_sum[:])
                            bc = pp1.tile([128, PANEL], f32, name=f"bc_ps{h}_{p}", tag="bc_ps", bufs=1)
                            nc.tensor.matmul(bc[:], orow[:], rec[:], start=True, stop=True)
                            bc_sb = tp.tile([128, PANEL], f32, name=f"bc_sb{h}_{p}", tag="bc_sb", bufs=2)
                            nc.vector.tensor_copy(bc_sb[:], bc[:])
                            at_p = tp.tile([128, PANEL], dt, name=f"at_p{h}_{p}", tag="at_p", bufs=2)
                            nc.vector.tensor_mul(at_p[:], ps_at[:], bc_sb[:])
                            for r in range(2):
                                jdx = 2 * p + r
                                nc.scalar.dma_start(
                                    out=a2a_in[h][jdx * VD:(jdx + 1) * VD, :],
                                    in_=at_p[:, SL * r:SL * (r + 1)])

                            if p == NPANEL - 1 and h == 0:
                                # h0 fully done: fire its AllToAll now so the
                                # flight overlaps h1's last panel + Wo-h0
                                nc.gpsimd.collective_compute(
                                    "AllToAll", ALU.bypass,
                                    replica_groups=[list(range(NC_))],
                                    ins=[a2a_in[0][:]], outs=[a2a_out[0][:]],
                                )
                                for jdx in range(NC_):
                                    c = 2 * jdx
                                    t = agp.tile([128, SL], dt, name=f"att_g{c}")
                                    nc.sync.dma_start(out=t[:], in_=a2a_out[0][128 * jdx:128 * (jdx + 1), :])
                                    att_g[c] = t

                    nc.gpsimd.collective_compute(
                        "AllToAll", ALU.bypass,
                        replica_groups=[list(range(NC_))],
                        ins=[a2a_in[1][:]], outs=[a2a_out[1][:]],
                    )
                    for jdx in range(NC_):
                        c = 2 * jdx + 1
                        t = agp.tile([128, SL], dt, name=f"att_g{c}")
                        nc.sync.dma_start(out=t[:], in_=a2a_out[1][128 * jdx:128 * (jdx + 1), :])
                        att_g[c] = t

                # ---- Wo in two halves: h0 during A2A-h1 flight, park PSUM ----
                with tc.tile_pool(name="wo_ps", bufs=1, space="PSUM") as wop, \
                     tc.tile_pool(name="wo_tmp", bufs=3) as wtp:
                    o_ps = {}
                    for col in range(HID // 512):
                        for qb in range(SL // 128):
                            ps = wop.tile([128, 512], f32, name=f"o_ps{col}_{qb}")
                            o_ps[(col, qb)] = ps
                            for c in range(0, HCH, 2):
                                nc.tensor.matmul(ps[:], att_g[c][:, 128 * qb:128 * (qb + 1)],
                                                 wo_t[c][:, 512 * col:512 * (col + 1)],
                                                 start=(c == 0), stop=False)
                    for col in range(HID // 512):
                        for qb in range(SL // 128):
                            ps = o_ps[(col, qb)]
                            for c in range(1, HCH, 2):
                                nc.tensor.matmul(ps[:], att_g[c][:, 128 * qb:128 * (qb + 1)],
                                                 wo_t[c][:, 512 * col:512 * (col + 1)],
                                                 start=False, stop=(c == HCH - 1))
                            osb = wtp.tile([128, 512], f32, name=f"osb{col}_{qb}", tag="osb", bufs=3)
                            nc.vector.tensor_copy(osb[:], ps[:])
                            nc.sync.dma_start(out=out_loc[128 * qb:128 * (qb + 1), 512 * col:512 * (col + 1)], in_=osb[:])

    nc.compile()
    return nc


def _to_dt(a, dt):
    if dt == bf16:
        return np.ascontiguousarray(a.astype(ml_dtypes.bfloat16))
    return np.ascontiguousarray(a.astype(np.float32))


def _pack_cols(mat, chunk_rows=128):
    """[R, C] -> [128, (R//128)*C]: row-chunk-major horizontal pack."""
    R, C = mat.shape
    n = R // chunk_rows
    return np.concatenate([mat[chunk_rows * i:chunk_rows * (i + 1), :] for i in range(n)], axis=1)


def _prepare_inputs(dt, hidden_states, position_ids, Wqa, qa_ln_w, Wqb, Wkva, kv_ln_w, Wkvb, Wo):
    perm = np.concatenate([np.arange(0, ROPE, 2), np.arange(1, ROPE, 2)])
    X = np.asarray(hidden_states, np.float32).reshape(S, HID)
    pos = np.asarray(position_ids).reshape(S).astype(np.float32)
    Wqa = np.asarray(Wqa, np.float32)
    Wkva = np.asarray(Wkva, np.float32)
    wqb_base = np.asarray(Wqb, np.float32) * np.asarray(qa_ln_w, np.float32)[:, None]
    wkvb_base = np.asarray(Wkvb, np.float32) * np.asarray(kv_ln_w, np.float32)[:, None]
    Wo = np.asarray(Wo, np.float32)

    # host rope tables (deinterleaved layout), transposed to [64, S]
    inv = (1.0 / (THETA ** (np.arange(0, ROPE, 2, dtype=np.float32) / ROPE))).astype(np.float32)
    freqs = np.concatenate([inv, inv])                     # (64,)
    emb = pos[:, None] * freqs[None, :]                    # (S, 64)
    sin_all = np.ascontiguousarray(np.sin(emb).T.astype(np.float32))   # (64, S)
    cos_all = np.ascontiguousarray(np.cos(emb).T.astype(np.float32))

    # wa_kv packed: per hid-chunk [kv 512 | pe(deint) 64]
    wkva_kv = Wkva[:, :KVLR]
    wkva_pe = Wkva[:, KVLR:][:, perm]
    wa_kv = _pack_cols(np.concatenate([wkva_kv, wkva_pe], axis=1))     # [128, 16*576]
    wa_q = _pack_cols(Wqa)                                             # [128, 16*1536]

    head_blocks = []
    for h in range(NH):
        cols = wqb_base[:, 192 * h:192 * (h + 1)]
        nope = cols[:, :NOPE]
        pe_d = cols[:, NOPE:][:, perm]
        rot = np.concatenate([-pe_d[:, 32:], pe_d[:, :32]], axis=1)
        head_blocks.append(np.concatenate([nope, pe_d, rot], axis=1))  # (1536, 256)
    k_blocks = [wkvb_base[:, 256 * h:256 * h + NOPE] for h in range(NH)]
    v_blocks = [wkvb_base[:, 256 * h + NOPE:256 * (h + 1)] for h in range(NH)]

    wa_kv_d = _to_dt(wa_kv, dt)
    wa_q_d = _to_dt(wa_q, dt)
    wo_d = _to_dt(Wo, dt)
    ones_col_d = _to_dt(np.ones((128, 1), np.float32), dt)
    ones_row_np = np.ones((1, 128), np.float32)

    in_maps = []
    for c in range(NC_):
        rows = slice(SL * c, SL * (c + 1))
        wqb_core = np.concatenate([head_blocks[HPC * c + h] for h in range(HPC)], axis=1)   # (1536, 512)
        wkk_core = np.concatenate([k_blocks[HPC * c + h] for h in range(HPC)], axis=1)      # (512, 256)
        wkv_core = np.concatenate([v_blocks[HPC * c + h] for h in range(HPC)], axis=1)      # (512, 256)
        in_maps.append({
            "x_t": _to_dt(_pack_cols(np.ascontiguousarray(X[rows, :].T)), dt),
            "wa_kv": wa_kv_d,
            "wa_q": wa_q_d,
            "wqb": _to_dt(_pack_cols(wqb_core), dt),
            "wkvb_k": _to_dt(_pack_cols(wkk_core), dt),
            "wkvb_v": _to_dt(_pack_cols(wkv_core), dt),
            "wo": wo_d,
            "ones_col": ones_col_d,
            "ones_row": ones_row_np,
            "sin_all": sin_all,
            "cos_all": cos_all,
            "sin_loc": np.ascontiguousarray(sin_all[:, rows]),
            "cos_loc": np.ascontiguousarray(cos_all[:, rows]),
        })
    return in_maps


def run(inputs, trace=False, trace_cores=None, dt=None):
    dt = dt if dt is not None else DT
    key = ("nc", str(dt))
    if key not in _CACHE:
        _CACHE[key] = build_program(dt)
    nc = _CACHE[key]
    in_maps = _prepare_inputs(dt, **inputs)
    res = run_bass_kernel_spmd(nc, in_maps, list(range(NC_)), trace=trace,
                               trace_cores=trace_cores)
    out = np.concatenate([res.results[c]["out_loc"] for c in range(NC_)], axis=0)
    return out.reshape(1, S, HID), res


def kernel(**inputs) -> np.ndarray:
    out, _ = run(inputs, trace=False)
    return out


# revision 16
# speedup vs baseline: 1.1331x; 1.0024x over previous
"""DeepseekV3 MLA flash-attention prefill kernel for 8 Trainium2 NeuronCores.

Sharding (SPMD, one program for all 8 cores):
  Stage A (sequence-parallel): core c computes low-rank down-projections
    X @ [Wqa|Wkva] for its 256 rows in transposed layout and AllGathers
    the UNNORMALIZED chunks plus the raw sum-of-squares rows (kv first,
    early trigger; q second). RMS scales are recovered cheaply in stage B
    (sqrt + fast-reciprocal) and folded into the K^T/V/q eviction
    multiplies, so nothing numeric sits on the AllGather trigger path.
  Stage B (head-parallel): core c owns heads {2c, 2c+1}: K^T/V built while
    the q AllGather is in flight; q projection and causal attention are
    interleaved per 512-wide panel; softmax runs in (k, q) layout without
    max subtraction; adjacent full k-blocks share one [128,1024] exp;
    diagonal blocks only compute/exp the live columns (affine_select
    zero-fills the rest); per-q normalization uses a fast DVE reciprocal
    plus a gpsimd partition-broadcast, double-buffered off critical path.
  Output: per-head AllToAll; Wo runs in two halves (h0 half during the
    h1 AllToAll flight, accumulating in parked PSUM banks).
"""

import sys

if '/opt/trn_rl_repo' not in sys.path:
    sys.path.insert(0, '/opt/trn_rl_repo')

import numpy as np
import ml_dtypes

import concourse.bass as bass
import concourse.mybir as mybir
import concourse.tile as tile
from concourse import bacc
from concourse.bass_utils import run_bass_kernel_spmd

f32 = mybir.dt.float32
f32r = mybir.dt.float32r
bf16 = mybir.dt.bfloat16
AF = mybir.ActivationFunctionType
ALU = mybir.AluOpType

NC_ = 8            # cores
S = 2048           # sequence
HID = 2048
QLR = 1536         # q lora rank
KVLR = 512         # kv lora rank
ROPE = 64
NOPE = 128
VD = 128
NH = 16
HPC = NH // NC_    # heads per core = 2
SL = S // NC_      # rows per core = 256
PANEL = 512        # q panel width
NPANEL = S // PANEL
NKB = S // 128     # 16 k blocks
QCH = QLR // 128   # 12
KCH = KVLR // 128  # 4
HCH = HID // 128   # 16
THETA = 10000.0
SM_SCALE = float((NOPE + ROPE) ** -0.5)

DT = bf16          # matmul dtype

NWG = 8            # wa_q DMA sub-groups (trickle arrival keeps HAM warm)
WARM1 = 10         # keep-warm links during AG-q wait

KV_ROWS = KCH * 128 + ROPE + 1      # ckv | k_pe | ssq row  = 577
Q_ROWS = QCH * 128 + 1              # q_a | ssq row         = 1537

_CACHE = {}


def build_program(dt):
    nc = bacc.Bacc("TRN2", target_bir_lowering=False, debug=False, num_devices=NC_)

    def din(name, shape, d=None):
        return nc.dram_tensor(name, shape, d or dt, kind="ExternalInput")

    x_t = din("x_t", [128, HCH * SL])                   # hc-major pack
    wa_kv = din("wa_kv", [128, HCH * (KVLR + ROPE)])    # per hc [kv|pe]
    wa_q = din("wa_q", [128, HCH * QLR])
    wqb = din("wqb", [128, QCH * HPC * 256])            # per l [nope|pe|rot]x2
    wkvb_k = din("wkvb_k", [128, KCH * HPC * NOPE])
    wkvb_v = din("wkvb_v", [128, KCH * HPC * VD])
    wo = din("wo", [128, HCH * HID])                    # per c rows pack
    ones_col = din("ones_col", [128, 1])
    sin_loc = din("sin_loc", [ROPE, SL])
    cos_loc = din("cos_loc", [ROPE, SL])
    sin_all = din("sin_all", [ROPE, S])
    cos_all = din("cos_all", [ROPE, S])
    out_loc = nc.dram_tensor("out_loc", [SL, HID], f32, kind="ExternalOutput")

    HG = HCH // 4          # hc per wa_kv/x group
    QHG = HCH // NWG       # hc per wa_q group

    with tile.TileContext(nc) as tc:
        with tc.tile_pool(name="dram", bufs=1, space="DRAM") as dpool, \
             tc.tile_pool(name="consts", bufs=1) as cpool:
            ag_in_kv = dpool.tile([KV_ROWS, SL], dt)
            ag_out_kv = dpool.tile([NC_ * KV_ROWS, SL], dt, addr_space="Shared")
            ag_in_q = dpool.tile([Q_ROWS, SL], dt)
            ag_out_q = dpool.tile([NC_ * Q_ROWS, SL], dt, addr_space="Shared")
            a2a_in = [dpool.tile([NC_ * VD, SL], dt, name=f"a2a_in{h}") for h in range(HPC)]
            a2a_out = [dpool.tile([NC_ * VD, SL], dt, name=f"a2a_out{h}") for h in range(HPC)]

            ocol = cpool.tile([128, 1], dt)
            sin_l = cpool.tile([ROPE, SL], dt)
            cos_l = cpool.tile([ROPE, SL], dt)
            sin_g = cpool.tile([ROPE, S], dt)
            cos_g = cpool.tile([ROPE, S], dt)
            # stage-B weights hoisted to never-freed space so their loads
            # carry no WAR dependency on stage-A pools
            wkk_t = cpool.tile([128, KCH * HPC * NOPE], dt)
            wkv_t = cpool.tile([128, KCH * HPC * VD], dt)
            wqb_t = cpool.tile([128, QCH * HPC * 256], dt)
            gpboot = cpool.tile([1, 16], f32)

            # kick the gpsimd engine boot at t=0 (its first instruction
            # otherwise lands right before the first collective trigger)
            nc.gpsimd.memset(gpboot[:], 0.0)

            nc.sync.dma_start(out=ocol[:], in_=ones_col[:])
            nc.scalar.dma_start(out=sin_l[:], in_=sin_loc[:])
            nc.scalar.dma_start(out=cos_l[:], in_=cos_loc[:])
            nc.scalar.dma_start(out=wkk_t[:], in_=wkvb_k[:])
            nc.scalar.dma_start(out=wkv_t[:], in_=wkvb_v[:])
            nc.scalar.dma_start(out=wqb_t[:], in_=wqb[:])
            nc.scalar.dma_start(out=sin_g[:], in_=sin_all[:])
            nc.scalar.dma_start(out=cos_g[:], in_=cos_all[:])

            # ================= Stage A: transposed down projections =================
            with tc.tile_pool(name="sa_x", bufs=1) as xp, \
                 tc.tile_pool(name="sa_w", bufs=1) as wp, \
                 tc.tile_pool(name="sa_tmp", bufs=2) as tp, \
                 tc.tile_pool(name="sa_ps", bufs=2, space="PSUM") as pp, \
                 tc.tile_pool(name="sa_ps1", bufs=1, space="PSUM") as pp1:

                x_g = []
                wakv_g = []
                for g in range(4):
                    xt = xp.tile([128, HG * SL], dt, name=f"xg{g}")
                    nc.sync.dma_start(out=xt[:], in_=x_t[:, HG * SL * g:HG * SL * (g + 1)])
                    x_g.append(xt)
                    wt = wp.tile([128, HG * 576], dt, name=f"wakv{g}")
                    nc.sync.dma_start(out=wt[:], in_=wa_kv[:, HG * 576 * g:HG * 576 * (g + 1)])
                    wakv_g.append(wt)
                waq_g = []
                for g in range(NWG):
                    wt = wp.tile([128, QHG * QLR], dt, name=f"waq{g}")
                    nc.sync.dma_start(out=wt[:], in_=wa_q[:, QHG * QLR * g:QHG * QLR * (g + 1)])
                    waq_g.append(wt)

                def xs(hc):
                    return x_g[hc // HG][:, SL * (hc % HG):SL * (hc % HG + 1)]

                # kv chunks: raw cast to AG staging + ssq accumulate
                ssq_kv = pp1.tile([1, SL], f32, name="ssq_kv")
                prev_sq = None
                for o in range(KCH):
                    ps = pp.tile([128, SL], f32, name=f"ps_kv{o}", tag="a_ps", bufs=2)
                    for hc in range(HCH):
                        w = wakv_g[hc // HG]
                        c0 = 576 * (hc % HG) + 128 * o
                        nc.tensor.matmul(ps[:], w[:, c0:c0 + 128], xs(hc),
                                         start=(hc == 0), stop=(hc == HCH - 1))
                    if prev_sq is not None:
                        nc.tensor.matmul(ssq_kv[:], ocol[:], prev_sq[:], start=(o == 1), stop=False)
                    cast = tp.tile([128, SL], dt, name=f"kvc{o}", tag="cast", bufs=3)
                    nc.vector.tensor_copy(cast[:], ps[:])
                    nc.scalar.dma_start(out=ag_in_kv[128 * o:128 * (o + 1), :], in_=cast[:])
                    sq = tp.tile([128, SL], dt, name=f"sqk{o}", tag="sq", bufs=2)
                    nc.scalar.activation(sq[:], ps[:], AF.Square)
                    prev_sq = sq

                # pe chunk + rope (host sin/cos slices); k_pe is not rms-normed
                ps_pe = pp.tile([ROPE, SL], f32, name="ps_pe", tag="a_ps", bufs=2)
                for hc in range(HCH):
                    w = wakv_g[hc // HG]
                    c0 = 576 * (hc % HG) + KVLR
                    nc.tensor.matmul(ps_pe[:], w[:, c0:c0 + ROPE], xs(hc),
                                     start=(hc == 0), stop=(hc == HCH - 1))
                nc.tensor.matmul(ssq_kv[:], ocol[:], prev_sq[:], start=False, stop=True)
                krot = tp.tile([ROPE, SL], f32, name="krot", tag="krot", bufs=1)
                nc.vector.tensor_scalar(out=krot[0:32, :], in0=ps_pe[32:64, :], scalar1=-1.0, scalar2=None, op0=ALU.mult)
                nc.vector.tensor_copy(krot[32:64, :], ps_pe[0:32, :])
                kro = tp.tile([ROPE, SL], f32, name="kro", tag="kro", bufs=1)
                nc.vector.tensor_mul(kro[:], ps_pe[:], cos_l[:])
                krs = tp.tile([ROPE, SL], f32, name="krs", tag="krs", bufs=1)
                nc.vector.tensor_mul(krs[:], krot[:], sin_l[:])
                kfin = tp.tile([ROPE, SL], dt, name="kfin", tag="kfin", bufs=1)
                nc.vector.tensor_add(kfin[:], kro[:], krs[:])
                nc.scalar.dma_start(out=ag_in_kv[KCH * 128:KCH * 128 + ROPE, :], in_=kfin[:])
                sqr_kv = tp.tile([1, SL], dt, name="sqr_kv", tag="sqr", bufs=2)
                nc.vector.tensor_copy(sqr_kv[:], ssq_kv[:])
                nc.scalar.dma_start(out=ag_in_kv[KV_ROWS - 1:KV_ROWS, :], in_=sqr_kv[:])

                nc.gpsimd.collective_compute(
                    "AllGather", ALU.bypass,
                    replica_groups=[list(range(NC_))],
                    ins=[ag_in_kv[:]], outs=[ag_out_kv[:]],
                )

                # q chunks (wa_q trickles in; ssq matmul lags one chunk)
                ssq_q = pp1.tile([1, SL], f32, name="ssq_q")
                prev_sq = None
                for o in range(QCH):
                    ps = pp.tile([128, SL], f32, name=f"ps_q{o}", tag="a_ps", bufs=2)
                    for hc in range(HCH):
                        w = waq_g[hc // QHG]
                        c0 = QLR * (hc % QHG) + 128 * o
                        nc.tensor.matmul(ps[:], w[:, c0:c0 + 128], xs(hc),
                                         start=(hc == 0), stop=(hc == HCH - 1))
                    if prev_sq is not None:
                        nc.tensor.matmul(ssq_q[:], ocol[:], prev_sq[:], start=(o == 1), stop=False)
                    cast = tp.tile([128, SL], dt, name=f"qc{o}", tag="cast", bufs=3)
                    nc.vector.tensor_copy(cast[:], ps[:])
                    nc.scalar.dma_start(out=ag_in_q[128 * o:128 * (o + 1), :], in_=cast[:])
                    sq = tp.tile([128, SL], dt, name=f"sqq{o}", tag="sq", bufs=2)
                    nc.scalar.activation(sq[:], ps[:], AF.Square)
                    prev_sq = sq
                nc.tensor.matmul(ssq_q[:], ocol[:], prev_sq[:], start=False, stop=True)
                sqr_q = tp.tile([1, SL], dt, name="sqr_q", tag="sqr", bufs=2)
                nc.vector.tensor_copy(sqr_q[:], ssq_q[:])
                nc.scalar.dma_start(out=ag_in_q[Q_ROWS - 1:Q_ROWS, :], in_=sqr_q[:])

                nc.gpsimd.collective_compute(
                    "AllGather", ALU.bypass,
                    replica_groups=[list(range(NC_))],
                    ins=[ag_in_q[:]], outs=[ag_out_q[:]],
                )

            agkv_r = ag_out_kv.rearrange("(r c) q -> r c q", r=NC_)
            agq_r = ag_out_q.rearrange("(r c) q -> r c q", r=NC_)

            # ================= Stage B: head-parallel attention =================
            with tc.tile_pool(name="sb_res", bufs=1) as rp, \
                 tc.tile_pool(name="sb_wo", bufs=24) as wsp, \
                 tc.tile_pool(name="sb_ag", bufs=1) as agp:
                att_g = {}
                wo_s = {}
                qn_sb = {}
                qp_sb = {}

                with tc.tile_pool(name="sb_qa", bufs=2) as qap, \
                     tc.tile_pool(name="sb_tmp", bufs=2) as tp, \
                     tc.tile_pool(name="sb_pt", bufs=3) as ptp, \
                     tc.tile_pool(name="sb_ps", bufs=2, space="PSUM") as pp, \
                     tc.tile_pool(name="sb_psat", bufs=2, space="PSUM") as ppat, \
                     tc.tile_pool(name="sb_ps1", bufs=1, space="PSUM") as pp1:

                    # gathered kv rows (sync queue)
                    kpe_g = rp.tile([ROPE, S], dt, name="kpe_g")
                    ssqkv_g = tp.tile([1, S], dt, name="ssqkv_g", tag="ssqg", bufs=1)
                    for r in range(NC_):
                        nc.sync.dma_start(out=kpe_g[:, SL * r:SL * (r + 1)],
                                          in_=agkv_r[r, KCH * 128:KCH * 128 + ROPE, :])
                        nc.sync.dma_start(out=ssqkv_g[:, SL * r:SL * (r + 1)],
                                          in_=agkv_r[r, KV_ROWS - 1:KV_ROWS, :])
                    # rkv row: sqrt(ssq/512) then fast reciprocal
                    mkv = tp.tile([1, S], f32, name="mkv", tag="mrow", bufs=1)
                    nc.scalar.activation(mkv[:], ssqkv_g[:], AF.Sqrt, scale=1.0 / KVLR)
                    rkv_row = tp.tile([1, S], f32, name="rkv_row", tag="rrow", bufs=1)
                    nc.vector.reciprocal_approx_fast(out=rkv_row[:], in_=mkv[:])
                    rkv_bf = rp.tile([1, S], dt, name="rkv_bf")  # kept: feeds PE transposes
                    nc.vector.tensor_copy(rkv_bf[:], rkv_row[:])
                    bc_rkv = rp.tile([128, S], dt, name="bc_rkv")
                    nc.gpsimd.partition_broadcast(bc_rkv[:], rkv_bf[:])
                    # rkv as columns for the V scale: 16x [1,128]->[128,1] via PE
                    ps_col = pp.tile([128, 1024], f32, name="ps_col", tag="mm_ps", bufs=2)
                    for kb in range(NKB):
                        nc.tensor.matmul(ps_col[:, kb:kb + 1], rkv_bf[0:1, 128 * kb:128 * (kb + 1)],
                                         ocol[0:1, 0:1], start=True, stop=True)
                    rkv_col = rp.tile([128, NKB], f32, name="rkv_col")
                    nc.vector.tensor_copy(rkv_col[:], ps_col[:, 0:NKB])

                    kT = [rp.tile([128, S], dt, name=f"kT{h}") for h in range(HPC)]
                    v_t = [rp.tile([128, HPC * VD], dt, name=f"v_t{kb}") for kb in range(NKB)]
                    with tc.tile_pool(name="sb_ckv", bufs=1) as ckvp:
                        ckv_g = []
                        for j in range(KCH):
                            t = ckvp.tile([128, S], dt, name=f"ckv_g{j}")
                            for r in range(NC_):
                                nc.sync.dma_start(out=t[:, SL * r:SL * (r + 1)],
                                                  in_=agkv_r[r, 128 * j:128 * (j + 1), :])
                            ckv_g.append(t)
                        for h in range(HPC):
                            for kc in range(S // 512):
                                ps = pp.tile([128, 1024], f32, name=f"kt_ps{h}_{kc}", tag="mm_ps", bufs=2)
                                for l in range(KCH):
                                    nc.tensor.matmul(ps[:, 0:512],
                                                     wkk_t[:, 256 * l + NOPE * h:256 * l + NOPE * (h + 1)],
                                                     ckv_g[l][:, 512 * kc:512 * (kc + 1)],
                                                     start=(l == 0), stop=(l == KCH - 1))
                                nc.vector.tensor_mul(kT[h][:, 512 * kc:512 * (kc + 1)], ps[:, 0:512],
                                                     bc_rkv[:, 512 * kc:512 * (kc + 1)])
                        for kq in range(NKB // 4):
                            ps = pp.tile([128, 1024], f32, name=f"v_ps{kq}", tag="mm_ps", bufs=2)
                            for i in range(4):
                                kb = 4 * kq + i
                                for l in range(KCH):
                                    nc.tensor.matmul(ps[:, 256 * i:256 * (i + 1)],
                                                     ckv_g[l][:, 128 * kb:128 * (kb + 1)],
                                                     wkv_t[:, 256 * l:256 * (l + 1)],
                                                     start=(l == 0), stop=(l == KCH - 1))
                            for i in range(4):
                                kb = 4 * kq + i
                                nc.vector.tensor_scalar(out=v_t[kb][:], in0=ps[:, 256 * i:256 * (i + 1)],
                                                        scalar1=rkv_col[:, kb:kb + 1], scalar2=None,
                                                        op0=ALU.mult)

                    # keep-warm chain during AG-q flight
                    warm_sb = tp.tile([128, 512], dt, name="warm_sb", tag="warm", bufs=1)
                    nc.vector.tensor_copy(warm_sb[:], kT[0][:, 0:512])
                    warm_ps = pp1.tile([1, PANEL], f32, name="warm_ps", tag="sum_ps", bufs=2)
                    wfb = tp.tile([1, 16], f32, name="wfb", tag="wfb", bufs=1)
                    for i in range(WARM1):
                        nc.tensor.matmul(warm_ps[:], ocol[:], warm_sb[:], start=True, stop=True)
                        nc.vector.tensor_copy(wfb[:], warm_ps[:, 0:16])
                        nc.vector.tensor_copy(warm_sb[0:1, 0:16], wfb[:])

                    # rq row from the gathered q ssq
                    ssqq_g = tp.tile([1, S], dt, name="ssqq_g", tag="ssqg", bufs=1)
                    for r in range(NC_):
                        nc.sync.dma_start(out=ssqq_g[:, SL * r:SL * (r + 1)],
                                          in_=agq_r[r, Q_ROWS - 1:Q_ROWS, :])
                    mq = tp.tile([1, S], f32, name="mq", tag="mrow", bufs=1)
                    nc.scalar.activation(mq[:], ssqq_g[:], AF.Sqrt, scale=1.0 / QLR)
                    rq_row = tp.tile([1, S], f32, name="rq_row", tag="rrow", bufs=1)
                    nc.vector.reciprocal_approx_fast(out=rq_row[:], in_=mq[:])
                    rq_bf = rp.tile([1, S], dt, name="rq_bf")
                    nc.vector.tensor_copy(rq_bf[:], rq_row[:])

                    # ---- per panel: q projection then attention (both heads) ----
                    for p in range(NPANEL):
                        qs = slice(PANEL * p, PANEL * (p + 1))
                        bc_rq = tp.tile([128, PANEL], dt, name=f"bc_rq{p}", tag="bc_rq", bufs=2)
                        nc.gpsimd.partition_broadcast(bc_rq[:], rq_bf[0:1, qs])
                        qa_p = []
                        for l in range(QCH):
                            t = qap.tile([128, PANEL], dt, name=f"qa_p{p}_{l}", tag=f"qa_p{l}", bufs=2)
                            for r in range(2):
                                nc.sync.dma_start(out=t[:, SL * r:SL * (r + 1)],
                                                  in_=agq_r[2 * p + r, 128 * l:128 * (l + 1), :])
                            qa_p.append(t)
                        for h in range(HPC):
                            hcol = 256 * h
                            ps_qnr = pp.tile([128, 1024], f32, name=f"qnr_ps{h}_{p}", tag="mm_ps", bufs=2)
                            for l in range(QCH):
                                nc.tensor.matmul(ps_qnr[:, 0:512], wqb_t[:, 512 * l + hcol:512 * l + hcol + NOPE],
                                                 qa_p[l][:], start=(l == 0), stop=(l == QCH - 1))
                            for l in range(QCH):
                                nc.tensor.matmul(ps_qnr[:, 512:1024], wqb_t[:, 512 * l + hcol + NOPE:512 * l + hcol + 256],
                                                 qa_p[l][:], start=(l == 0), stop=(l == QCH - 1))
                            qn = tp.tile([128, PANEL], dt, name=f"qn_sb{h}_{p}", tag="qn", bufs=4)
                            nc.vector.tensor_mul(qn[:], ps_qnr[:, 0:512], bc_rq[:])
                            qn_sb[(h, p)] = qn
                            qt1 = tp.tile([ROPE, PANEL], f32, name=f"qt1_{h}_{p}", tag="qt1", bufs=1)
                            nc.vector.tensor_mul(qt1[:], ps_qnr[0:ROPE, 512:1024], cos_g[:, qs])
                            qt2 = tp.tile([ROPE, PANEL], f32, name=f"qt2_{h}_{p}", tag="qt2", bufs=1)
                            nc.vector.tensor_mul(qt2[:], ps_qnr[ROPE:2 * ROPE, 512:1024], sin_g[:, qs])
                            qpt = tp.tile([ROPE, PANEL], f32, name=f"qpt_{h}_{p}", tag="qpt", bufs=1)
                            nc.vector.tensor_add(qpt[:], qt1[:], qt2[:])
                            qp = tp.tile([ROPE, PANEL], dt, name=f"qp_sb{h}_{p}", tag="qp", bufs=4)
                            nc.vector.tensor_mul(qp[:], qpt[:], bc_rq[0:ROPE, :])
                            qp_sb[(h, p)] = qp

                        for h in range(HPC):
                            nkb = 4 * (p + 1)
                            ps_at = ppat.tile([128, PANEL], f32, name=f"at_ps{h}_{p}", tag="at_ps", bufs=2)
                            ps_sum = pp1.tile([1, PANEL], f32, name=f"sum_ps{h}_{p}", tag="sum_ps", bufs=2)
                            pts = {}
                            pending = []

                            def consume(kb, nkb=nkb, ps_sum=ps_sum, ps_at=ps_at, pts=pts, h=h):
                                t, c0 = pts[kb]
                                nc.tensor.matmul(ps_sum[:], ocol[:], t[:, c0:c0 + PANEL],
                                                 start=(kb == 0), stop=(kb == nkb - 1))
                                nc.tensor.matmul(ps_at[:], v_t[kb][:, VD * h:VD * (h + 1)], t[:, c0:c0 + PANEL],
                                                 start=(kb == 0), stop=(kb == nkb - 1))

                            def flush():
                                while pending:
                                    consume(pending.pop(0))

                            kb = 0
                            while kb < nkb:
                                if kb + 1 < 4 * p:
                                    # two full blocks share one [128,1024] exp
                                    ps_sc = pp.tile([128, 1024], f32, name=f"sc2_{h}_{p}_{kb}", tag="mm_ps", bufs=2)
                                    for i in range(2):
                                        b = kb + i
                                        cs = slice(512 * i, 512 * (i + 1))
                                        nc.tensor.matmul(ps_sc[:, cs], kT[h][:, 128 * b:128 * (b + 1)],
                                                         qn_sb[(h, p)][:], start=True, stop=False)
                                        nc.tensor.matmul(ps_sc[:, cs], kpe_g[:, 128 * b:128 * (b + 1)],
                                                         qp_sb[(h, p)][:], start=False, stop=True)
                                    pt = ptp.tile([128, 1024], dt, name=f"pt{h}_{p}_{kb}", tag="pt", bufs=3)
                                    nc.scalar.activation(pt[:], ps_sc[:], AF.Exp, scale=SM_SCALE)
                                    pts[kb] = (pt, 0)
                                    pts[kb + 1] = (pt, 512)
                                    nxt = [kb, kb + 1]
                                    kb += 2
                                else:
                                    j = kb - 4 * p
                                    c0 = 128 * j if j > 0 else 0
                                    ps_sc = pp.tile([128, 1024], f32, name=f"sc1_{h}_{p}_{kb}", tag="mm_ps", bufs=2)
                                    nc.tensor.matmul(ps_sc[:, c0:PANEL], kT[h][:, 128 * kb:128 * (kb + 1)],
                                                     qn_sb[(h, p)][:, c0:PANEL], start=True, stop=False)
                                    nc.tensor.matmul(ps_sc[:, c0:PANEL], kpe_g[:, 128 * kb:128 * (kb + 1)],
                                                     qp_sb[(h, p)][:, c0:PANEL], start=False, stop=True)
                                    pt = ptp.tile([128, 1024], dt, name=f"pt{h}_{p}_{kb}", tag="pt", bufs=3)
                                    nc.scalar.activation(pt[:, c0:PANEL], ps_sc[:, c0:PANEL], AF.Exp, scale=SM_SCALE)
                                    if j >= 0:
                                        nc.gpsimd.affine_select(
                                            out=pt[:, 0:PANEL], in_=pt[:, 0:PANEL],
                                            pattern=[[1, PANEL]],
                                            compare_op=ALU.is_ge,
                                            fill=0.0,
                                            base=-128 * j,
                                            channel_multiplier=-1)
                                    pts[kb] = (pt, 0)
                                    nxt = [kb]
                                    kb += 1
                                flush()
                                pending.extend(nxt)
                            flush()

                            rec = tp.tile([1, PANEL], f32, name=f"rec{h}_{p}", tag="rec", bufs=2)
                            nc.vector.reciprocal_approx_fast(out=rec[:], in_=ps_sum[:])
                            bc_sb = tp.tile([128, PANEL], f32, name=f"bc_sb{h}_{p}", tag="bc_sb", bufs=2)
                            nc.gpsimd.partition_broadcast(bc_sb[:], rec[:])
                            at_p = tp.tile([128, PANEL], dt, name=f"at_p{h}_{p}", tag="at_p", bufs=2)
                            nc.vector.tensor_mul(at_p[:], ps_at[:], bc_sb[:])
                            for r in range(2):
                                jdx = 2 * p + r
                                nc.scalar.dma_start(
                                    out=a2a_in[h][jdx * VD:(jdx + 1) * VD, :],
                                    in_=at_p[:, SL * r:SL * (r + 1)])

                            if p == NPANEL - 1 and h == 0:
                                nc.gpsimd.collective_compute(
                                    "AllToAll", ALU.bypass,
                                    replica_groups=[list(range(NC_))],
                                    ins=[a2a_in[0][:]], outs=[a2a_out[0][:]],
                                )
                                for jdx in range(NC_):
                                    c = 2 * jdx
                                    t = agp.tile([128, SL], dt, name=f"att_g{c}")
                                    nc.sync.dma_start(out=t[:], in_=a2a_out[0][128 * jdx:128 * (jdx + 1), :])
                                    att_g[c] = t
                                for par in range(2):
                                    for col in range(HID // 512):
                                        for c in range(par, HCH, 2):
                                            t = wsp.tile([128, 512], dt, name=f"wo_s{c}_{col}", tag="wo_s", bufs=24)
                                            nc.scalar.dma_start(
                                                out=t[:],
                                                in_=wo[:, HID * c + 512 * col:HID * c + 512 * (col + 1)])
                                            wo_s[(c, col)] = t

                    nc.gpsimd.collective_compute(
                        "AllToAll", ALU.bypass,
                        replica_groups=[list(range(NC_))],
                        ins=[a2a_in[1][:]], outs=[a2a_out[1][:]],
                    )
                    for jdx in range(NC_):
                        c = 2 * jdx + 1
                        t = agp.tile([128, SL], dt, name=f"att_g{c}")
                        nc.sync.dma_start(out=t[:], in_=a2a_out[1][128 * jdx:128 * (jdx + 1), :])
                        att_g[c] = t

                # ---- Wo in two halves: h0 during A2A-h1 flight, park PSUM ----
                with tc.tile_pool(name="wo_ps", bufs=1, space="PSUM") as wop, \
                     tc.tile_pool(name="wo_tmp", bufs=3) as wtp:
                    o_ps = {}
                    for col in range(HID // 512):
                        for qb in range(SL // 128):
                            ps = wop.tile([128, 512], f32, name=f"o_ps{col}_{qb}")
                            o_ps[(col, qb)] = ps
                            for c in range(0, HCH, 2):
                                nc.tensor.matmul(ps[:], att_g[c][:, 128 * qb:128 * (qb + 1)],
                                                 wo_s[(c, col)][:], start=(c == 0), stop=False)
                    for col in range(HID // 512):
                        for qb in range(SL // 128):
                            ps = o_ps[(col, qb)]
                            for c in range(1, HCH, 2):
                                nc.tensor.matmul(ps[:], att_g[c][:, 128 * qb:128 * (qb + 1)],
                                                 wo_s[(c, col)][:], start=False, stop=(c == HCH - 1))
                            osb = wtp.tile([128, 512], f32, name=f"osb{col}_{qb}", tag="osb", bufs=3)
                            nc.vector.tensor_copy(osb[:], ps[:])
                            nc.sync.dma_start(out=out_loc[128 * qb:128 * (qb + 1), 512 * col:512 * (col + 1)], in_=osb[:])

    nc.compile()
    return nc


def _to_dt(a, dt):
    if dt == bf16:
        return np.ascontiguousarray(a.astype(ml_dtypes.bfloat16))
    return np.ascontiguousarray(a.astype(np.float32))


def _pack_cols(mat, chunk_rows=128):
    """[R, C] -> [128, (R//128)*C]: row-chunk-major horizontal pack."""
    R, C = mat.shape
    n = R // chunk_rows
    return np.concatenate([mat[chunk_rows * i:chunk_rows * (i + 1), :] for i in range(n)], axis=1)


def _prepare_inputs(dt, hidden_states, position_ids, Wqa, qa_ln_w, Wqb, Wkva, kv_ln_w, Wkvb, Wo):
    perm = np.concatenate([np.arange(0, ROPE, 2), np.arange(1, ROPE, 2)])
    X = np.asarray(hidden_states, np.float32).reshape(S, HID)
    pos = np.asarray(position_ids).reshape(S).astype(np.float32)
    Wqa = np.asarray(Wqa, np.float32)
    Wkva = np.asarray(Wkva, np.float32)
    wqb_base = np.asarray(Wqb, np.float32) * np.asarray(qa_ln_w, np.float32)[:, None]
    wkvb_base = np.asarray(Wkvb, np.float32) * np.asarray(kv_ln_w, np.float32)[:, None]
    Wo = np.asarray(Wo, np.float32)

    inv = (1.0 / (THETA ** (np.arange(0, ROPE, 2, dtype=np.float32) / ROPE))).astype(np.float32)
    freqs = np.concatenate([inv, inv])                     # (64,)
    emb = pos[:, None] * freqs[None, :]                    # (S, 64)
    sin_all = np.ascontiguousarray(np.sin(emb).T.astype(np.float32))   # (64, S)
    cos_all = np.ascontiguousarray(np.cos(emb).T.astype(np.float32))

    wkva_kv = Wkva[:, :KVLR]
    wkva_pe = Wkva[:, KVLR:][:, perm]
    wa_kv = _pack_cols(np.concatenate([wkva_kv, wkva_pe], axis=1))     # [128, 16*576]
    wa_q = _pack_cols(Wqa)                                             # [128, 16*1536]

    head_blocks = []
    for h in range(NH):
        cols = wqb_base[:, 192 * h:192 * (h + 1)]
        nope = cols[:, :NOPE]
        pe_d = cols[:, NOPE:][:, perm]
        rot = np.concatenate([-pe_d[:, 32:], pe_d[:, :32]], axis=1)
        head_blocks.append(np.concatenate([nope, pe_d, rot], axis=1))  # (1536, 256)
    k_blocks = [wkvb_base[:, 256 * h:256 * h + NOPE] for h in range(NH)]
    v_blocks = [wkvb_base[:, 256 * h + NOPE:256 * (h + 1)] for h in range(NH)]

    wa_kv_d = _to_dt(wa_kv, dt)
    wa_q_d = _to_dt(wa_q, dt)
    wo_d = _to_dt(_pack_cols(Wo), dt)                                  # [128, 16*2048]
    ones_col_d = _to_dt(np.ones((128, 1), np.float32), dt)

    in_maps = []
    for c in range(NC_):
        rows = slice(SL * c, SL * (c + 1))
        wqb_core = np.concatenate([head_blocks[HPC * c + h] for h in range(HPC)], axis=1)   # (1536, 512)
        wkk_core = np.concatenate([k_blocks[HPC * c + h] for h in range(HPC)], axis=1)      # (512, 256)
        wkv_core = np.concatenate([v_blocks[HPC * c + h] for h in range(HPC)], axis=1)      # (512, 256)
        in_maps.append({
            "x_t": _to_dt(_pack_cols(np.ascontiguousarray(X[rows, :].T)), dt),
            "wa_kv": wa_kv_d,
            "wa_q": wa_q_d,
            "wqb": _to_dt(_pack_cols(wqb_core), dt),
            "wkvb_k": _to_dt(_pack_cols(wkk_core), dt),
            "wkvb_v": _to_dt(_pack_cols(wkv_core), dt),
            "wo": wo_d,
            "ones_col": ones_col_d,
            "sin_all": _to_dt(sin_all, dt),
            "cos_all": _to_dt(cos_all, dt),
            "sin_loc": _to_dt(sin_all[:, rows], dt),
            "cos_loc": _to_dt(cos_all[:, rows], dt),
        })
    return in_maps


def run(inputs, trace=False, trace_cores=None, dt=None):
    dt = dt if dt is not None else DT
    key = ("nc", str(dt))
    if key not in _CACHE:
        _CACHE[key] = build_program(dt)
    nc = _CACHE[key]
    in_maps = _prepare_inputs(dt, **inputs)
    res = run_bass_kernel_spmd(nc, in_maps, list(range(NC_)), trace=trace,
                               trace_cores=trace_cores)
    out = np.concatenate([res.results[c]["out_loc"] for c in range(NC_)], axis=0)
    return out.reshape(1, S, HID), res


def kernel(**inputs) -> np.ndarray:
    out, _ = run(inputs, trace=False)
    return out


# revision 23
# speedup vs baseline: 1.2186x; 1.0755x over previous
"""DeepseekV3 MLA flash-attention prefill kernel for 8 Trainium2 NeuronCores.

Sharding (SPMD, one program for all 8 cores):
  Stage A (sequence-parallel): core c computes low-rank down-projections
    X @ [Wqa|Wkva] for its 256 rows in transposed layout and AllGathers
    the UNNORMALIZED chunks plus the raw sum-of-squares rows (kv first,
    early trigger; q second). RMS scales are recovered cheaply in stage B
    (sqrt + fast-reciprocal) and folded into the K^T/V/q eviction
    multiplies, so nothing numeric sits on the AllGather trigger path.
  Stage B (head-parallel): core c owns heads {2c, 2c+1}: K^T/V built while
    the q AllGather is in flight; q projection and causal attention are
    interleaved per 512-wide panel; softmax runs in (k, q) layout without
    max subtraction; adjacent full k-blocks share one [128,1024] exp;
    diagonal blocks only compute/exp the live columns (affine_select
    zero-fills the rest); per-q normalization uses a fast DVE reciprocal
    plus a gpsimd partition-broadcast, double-buffered off critical path.
  Output: per-head AllToAll; Wo runs in two halves (h0 half during the
    h1 AllToAll flight, accumulating in parked PSUM banks).
"""

import sys

if '/opt/trn_rl_repo' not in sys.path:
    sys.path.insert(0, '/opt/trn_rl_repo')

import numpy as np
import ml_dtypes

import concourse.bass as bass
import concourse.mybir as mybir
import concourse.tile as tile
from concourse import bacc
from concourse.bass_utils import run_bass_kernel_spmd

f32 = mybir.dt.float32
f32r = mybir.dt.float32r
bf16 = mybir.dt.bfloat16
AF = mybir.ActivationFunctionType
ALU = mybir.AluOpType

NC_ = 8            # cores
S = 2048           # sequence
HID = 2048
QLR = 1536         # q lora rank
KVLR = 512         # kv lora rank
ROPE = 64
NOPE = 128
VD = 128
NH = 16
HPC = NH // NC_    # heads per core = 2
SL = S // NC_      # rows per core = 256
PANEL = 512        # q panel width
NPANEL = S // PANEL
NKB = S // 128     # 16 k blocks
QCH = QLR // 128   # 12
KCH = KVLR // 128  # 4
HCH = HID // 128   # 16
THETA = 10000.0
SM_SCALE = float((NOPE + ROPE) ** -0.5)

DT = bf16          # matmul dtype

NWG = 8            # wa_q DMA sub-groups (trickle arrival keeps HAM warm)
WARM1 = 30         # keep-warm links during the combined-AG flight

KV_ROWS = KCH * 128 + ROPE          # ckv | k_pe
AG_ROWS = KV_ROWS + QCH * 128 + 2   # + q_a | ssq_kv | ssq_q = 2114
SSQKV_ROW = AG_ROWS - 2
SSQQ_ROW = AG_ROWS - 1

_CACHE = {}


def build_program(dt):
    nc = bacc.Bacc("TRN2", target_bir_lowering=False, debug=False, num_devices=NC_)

    def din(name, shape, d=None):
        return nc.dram_tensor(name, shape, d or dt, kind="ExternalInput")

    x_t = din("x_t", [128, HCH * SL])                   # hc-major pack
    wa_kv = din("wa_kv", [128, HCH * (KVLR + ROPE)])    # per hc [kv|pe]
    wa_q = din("wa_q", [128, HCH * QLR])
    wqb = din("wqb", [128, QCH * HPC * 256])            # per l [nope|pe|rot]x2
    wkvb_k = din("wkvb_k", [128, KCH * HPC * NOPE])
    wkvb_v = din("wkvb_v", [128, KCH * HPC * VD])
    wo = din("wo", [128, HCH * HID])                    # per c rows pack
    ones_col = din("ones_col", [128, 1])
    sin_loc = din("sin_loc", [ROPE, SL])
    cos_loc = din("cos_loc", [ROPE, SL])
    sin_all = din("sin_all", [ROPE, S])
    cos_all = din("cos_all", [ROPE, S])
    out_loc = nc.dram_tensor("out_loc", [SL, HID], f32, kind="ExternalOutput")

    HG = HCH // 4          # hc per wa_kv/x group
    QHG = HCH // NWG       # hc per wa_q group

    with tile.TileContext(nc) as tc:
        with tc.tile_pool(name="dram", bufs=1, space="DRAM") as dpool, \
             tc.tile_pool(name="consts", bufs=1) as cpool:
            agw_in = dpool.tile([128, 1], dt)
            agw_out = dpool.tile([NC_ * 128, 1], dt, addr_space="Shared")
            ag_in = dpool.tile([AG_ROWS, SL], dt)
            ag_out = dpool.tile([NC_ * AG_ROWS, SL], dt, addr_space="Shared")
            a2a_in = [dpool.tile([NC_ * VD, SL], dt, name=f"a2a_in{h}") for h in range(HPC)]
            a2a_out = [dpool.tile([NC_ * VD, SL], dt, name=f"a2a_out{h}") for h in range(HPC)]

            ocol = cpool.tile([128, 1], dt)
            sin_l = cpool.tile([ROPE, SL], dt)
            cos_l = cpool.tile([ROPE, SL], dt)
            sin_g = cpool.tile([ROPE, S], dt)
            cos_g = cpool.tile([ROPE, S], dt)
            # stage-B weights hoisted to never-freed space so their loads
            # carry no WAR dependency on stage-A pools
            wkk_t = cpool.tile([128, KCH * HPC * NOPE], dt)
            wkv_t = cpool.tile([128, KCH * HPC * VD], dt)
            wqb_t = cpool.tile([128, QCH * HPC * 256], dt)
            gpboot = cpool.tile([1, 16], f32)

            # kick the gpsimd boot at t=0; warm-up collective absorbs the
            # CC-init + first-collective latency while stage A computes
            nc.gpsimd.memset(gpboot[:], 0.0)
            nc.scalar.dma_start(out=agw_in[:], in_=ones_col[:])
            nc.gpsimd.collective_compute(
                "AllGather", ALU.bypass,
                replica_groups=[list(range(NC_))],
                ins=[agw_in[:]], outs=[agw_out[:]],
            )

            nc.sync.dma_start(out=ocol[:], in_=ones_col[:])
            nc.scalar.dma_start(out=sin_l[:], in_=sin_loc[:])
            nc.scalar.dma_start(out=cos_l[:], in_=cos_loc[:])
            nc.scalar.dma_start(out=wkk_t[:], in_=wkvb_k[:])
            nc.scalar.dma_start(out=wkv_t[:], in_=wkvb_v[:])
            nc.scalar.dma_start(out=wqb_t[:], in_=wqb[:])
            nc.scalar.dma_start(out=sin_g[:], in_=sin_all[:])
            nc.scalar.dma_start(out=cos_g[:], in_=cos_all[:])

            # ================= Stage A: transposed down projections =================
            with tc.tile_pool(name="sa_x", bufs=1) as xp, \
                 tc.tile_pool(name="sa_w", bufs=1) as wp, \
                 tc.tile_pool(name="sa_tmp", bufs=2) as tp, \
                 tc.tile_pool(name="sa_ps", bufs=2, space="PSUM") as pp, \
                 tc.tile_pool(name="sa_ps1", bufs=1, space="PSUM") as pp1:

                x_g = []
                wakv_g = []
                for g in range(4):
                    xt = xp.tile([128, HG * SL], dt, name=f"xg{g}")
                    nc.sync.dma_start(out=xt[:], in_=x_t[:, HG * SL * g:HG * SL * (g + 1)])
                    x_g.append(xt)
                    wt = wp.tile([128, HG * 576], dt, name=f"wakv{g}")
                    nc.sync.dma_start(out=wt[:], in_=wa_kv[:, HG * 576 * g:HG * 576 * (g + 1)])
                    wakv_g.append(wt)
                waq_g = []
                for g in range(NWG):
                    wt = wp.tile([128, QHG * QLR], dt, name=f"waq{g}")
                    nc.sync.dma_start(out=wt[:], in_=wa_q[:, QHG * QLR * g:QHG * QLR * (g + 1)])
                    waq_g.append(wt)

                def xs(hc):
                    return x_g[hc // HG][:, SL * (hc % HG):SL * (hc % HG + 1)]

                # kv chunks: raw cast to AG staging + ssq accumulate
                ssq_kv = pp1.tile([1, SL], f32, name="ssq_kv")
                prev_sq = None
                for o in range(KCH):
                    ps = pp.tile([128, SL], f32, name=f"ps_kv{o}", tag="a_ps", bufs=2)
                    for hc in range(HCH):
                        w = wakv_g[hc // HG]
                        c0 = 576 * (hc % HG) + 128 * o
                        nc.tensor.matmul(ps[:], w[:, c0:c0 + 128], xs(hc),
                                         start=(hc == 0), stop=(hc == HCH - 1))
                    if prev_sq is not None:
                        nc.tensor.matmul(ssq_kv[:], ocol[:], prev_sq[:], start=(o == 1), stop=False)
                    cast = tp.tile([128, SL], dt, name=f"kvc{o}", tag="cast", bufs=3)
                    nc.vector.tensor_copy(cast[:], ps[:])
                    nc.scalar.dma_start(out=ag_in[128 * o:128 * (o + 1), :], in_=cast[:])
                    sq = tp.tile([128, SL], dt, name=f"sqk{o}", tag="sq", bufs=2)
                    nc.scalar.activation(sq[:], ps[:], AF.Square)
                    prev_sq = sq

                # pe chunk + rope (host sin/cos slices); k_pe is not rms-normed
                ps_pe = pp.tile([ROPE, SL], f32, name="ps_pe", tag="a_ps", bufs=2)
                for hc in range(HCH):
                    w = wakv_g[hc // HG]
                    c0 = 576 * (hc % HG) + KVLR
                    nc.tensor.matmul(ps_pe[:], w[:, c0:c0 + ROPE], xs(hc),
                                     start=(hc == 0), stop=(hc == HCH - 1))
                nc.tensor.matmul(ssq_kv[:], ocol[:], prev_sq[:], start=False, stop=True)
                krot = tp.tile([ROPE, SL], f32, name="krot", tag="krot", bufs=1)
                nc.vector.tensor_scalar(out=krot[0:32, :], in0=ps_pe[32:64, :], scalar1=-1.0, scalar2=None, op0=ALU.mult)
                nc.vector.tensor_copy(krot[32:64, :], ps_pe[0:32, :])
                kro = tp.tile([ROPE, SL], f32, name="kro", tag="kro", bufs=1)
                nc.vector.tensor_mul(kro[:], ps_pe[:], cos_l[:])
                krs = tp.tile([ROPE, SL], f32, name="krs", tag="krs", bufs=1)
                nc.vector.tensor_mul(krs[:], krot[:], sin_l[:])
                kfin = tp.tile([ROPE, SL], dt, name="kfin", tag="kfin", bufs=1)
                nc.vector.tensor_add(kfin[:], kro[:], krs[:])
                nc.scalar.dma_start(out=ag_in[KCH * 128:KCH * 128 + ROPE, :], in_=kfin[:])
                sqr_kv = tp.tile([1, SL], dt, name="sqr_kv", tag="sqr", bufs=2)
                nc.vector.tensor_copy(sqr_kv[:], ssq_kv[:])
                nc.scalar.dma_start(out=ag_in[SSQKV_ROW:SSQKV_ROW + 1, :], in_=sqr_kv[:])

                # q chunks (wa_q trickles in; ssq matmul lags one chunk)
                ssq_q = pp1.tile([1, SL], f32, name="ssq_q")
                prev_sq = None
                for o in range(QCH):
                    ps = pp.tile([128, SL], f32, name=f"ps_q{o}", tag="a_ps", bufs=2)
                    for hc in range(HCH):
                        w = waq_g[hc // QHG]
                        c0 = QLR * (hc % QHG) + 128 * o
                        nc.tensor.matmul(ps[:], w[:, c0:c0 + 128], xs(hc),
                                         start=(hc == 0), stop=(hc == HCH - 1))
                    if prev_sq is not None:
                        nc.tensor.matmul(ssq_q[:], ocol[:], prev_sq[:], start=(o == 1), stop=False)
                    cast = tp.tile([128, SL], dt, name=f"qc{o}", tag="cast", bufs=3)
                    nc.vector.tensor_copy(cast[:], ps[:])
                    nc.scalar.dma_start(out=ag_in[KV_ROWS + 128 * o:KV_ROWS + 128 * (o + 1), :], in_=cast[:])
                    sq = tp.tile([128, SL], dt, name=f"sqq{o}", tag="sq", bufs=2)
                    nc.scalar.activation(sq[:], ps[:], AF.Square)
                    prev_sq = sq
                nc.tensor.matmul(ssq_q[:], ocol[:], prev_sq[:], start=False, stop=True)
                sqr_q = tp.tile([1, SL], dt, name="sqr_q", tag="sqr", bufs=2)
                nc.vector.tensor_copy(sqr_q[:], ssq_q[:])
                nc.scalar.dma_start(out=ag_in[SSQQ_ROW:SSQQ_ROW + 1, :], in_=sqr_q[:])

                nc.gpsimd.collective_compute(
                    "AllGather", ALU.bypass,
                    replica_groups=[list(range(NC_))],
                    ins=[ag_in[:]], outs=[ag_out[:]],
                )

                # keep-warm chain through the AG flight, seeded off the
                # last q cast so it starts when stage-A compute drains
                warm_sb2 = tp.tile([128, 512], dt, name="warm_sb2", tag="warm2", bufs=1)
                nc.vector.tensor_copy(warm_sb2[:, 0:256], cast[:])
                nc.vector.tensor_copy(warm_sb2[:, 256:512], cast[:])
                warm_ps2 = pp1.tile([1, 512], f32, name="warm_ps2")
                wfb2 = tp.tile([1, 16], f32, name="wfb2", tag="wfb2", bufs=1)
                for i in range(WARM1):
                    nc.tensor.matmul(warm_ps2[:], ocol[:], warm_sb2[:], start=True, stop=True)
                    nc.vector.tensor_copy(wfb2[:], warm_ps2[:, 0:16])
                    nc.vector.tensor_copy(warm_sb2[0:1, 0:16], wfb2[:])

            agr = ag_out.rearrange("(r c) q -> r c q", r=NC_)

            # ================= Stage B: head-parallel attention =================
            with tc.tile_pool(name="sb_res", bufs=1) as rp, \
                 tc.tile_pool(name="sb_wo", bufs=24) as wsp, \
                 tc.tile_pool(name="sb_ag", bufs=1) as agp:
                att_g = {}
                wo_s = {}
                qn_sb = {}
                qp_sb = {}

                with tc.tile_pool(name="sb_qa", bufs=2) as qap, \
                     tc.tile_pool(name="sb_tmp", bufs=2) as tp, \
                     tc.tile_pool(name="sb_acc", bufs=2) as accp, \
                     tc.tile_pool(name="sb_pt", bufs=3) as ptp, \
                     tc.tile_pool(name="sb_ps", bufs=2, space="PSUM") as pp, \
                     tc.tile_pool(name="sb_psat", bufs=2, space="PSUM") as ppat, \
                     tc.tile_pool(name="sb_ps1", bufs=1, space="PSUM") as pp1:

                    # gathered kv rows (sync queue)
                    kpe_g = rp.tile([ROPE, S], dt, name="kpe_g")
                    ssqkv_g = tp.tile([1, S], dt, name="ssqkv_g", tag="ssqg", bufs=1)
                    for r in range(NC_):
                        nc.sync.dma_start(out=kpe_g[:, SL * r:SL * (r + 1)],
                                          in_=agr[r, KCH * 128:KCH * 128 + ROPE, :])
                        nc.sync.dma_start(out=ssqkv_g[:, SL * r:SL * (r + 1)],
                                          in_=agr[r, SSQKV_ROW:SSQKV_ROW + 1, :])
                    # rkv row: sqrt(ssq/512) then fast reciprocal
                    mkv = tp.tile([1, S], f32, name="mkv", tag="mrow", bufs=1)
                    nc.scalar.activation(mkv[:], ssqkv_g[:], AF.Sqrt, scale=1.0 / KVLR)
                    rkv_row = tp.tile([1, S], f32, name="rkv_row", tag="rrow", bufs=1)
                    nc.vector.reciprocal_approx_fast(out=rkv_row[:], in_=mkv[:])
                    rkv_bf = rp.tile([1, S], dt, name="rkv_bf")  # kept: feeds PE transposes
                    nc.vector.tensor_copy(rkv_bf[:], rkv_row[:])
                    bc_rkv = rp.tile([128, S], dt, name="bc_rkv")
                    nc.gpsimd.partition_broadcast(bc_rkv[:], rkv_bf[:])
                    # rkv as columns for the V scale: 16x [1,128]->[128,1] via PE
                    ps_col = pp.tile([128, 1024], f32, name="ps_col", tag="mm_ps", bufs=2)
                    for kb in range(NKB):
                        nc.tensor.matmul(ps_col[:, kb:kb + 1], rkv_bf[0:1, 128 * kb:128 * (kb + 1)],
                                         ocol[0:1, 0:1], start=True, stop=True)
                    rkv_col = rp.tile([128, NKB], f32, name="rkv_col")
                    nc.vector.tensor_copy(rkv_col[:], ps_col[:, 0:NKB])

                    kT = [rp.tile([128, S], dt, name=f"kT{h}") for h in range(HPC)]
                    v_t = [rp.tile([128, HPC * VD], dt, name=f"v_t{kb}") for kb in range(NKB)]
                    with tc.tile_pool(name="sb_ckv", bufs=1) as ckvp:
                        ckv_g = []
                        for j in range(KCH):
                            t = ckvp.tile([128, S], dt, name=f"ckv_g{j}")
                            for r in range(NC_):
                                nc.sync.dma_start(out=t[:, SL * r:SL * (r + 1)],
                                                  in_=agr[r, 128 * j:128 * (j + 1), :])
                            ckv_g.append(t)
                        for h in range(HPC):
                            for kc in range(S // 512):
                                ps = pp.tile([128, 1024], f32, name=f"kt_ps{h}_{kc}", tag="mm_ps", bufs=2)
                                for l in range(KCH):
                                    nc.tensor.matmul(ps[:, 0:512],
                                                     wkk_t[:, 256 * l + NOPE * h:256 * l + NOPE * (h + 1)],
                                                     ckv_g[l][:, 512 * kc:512 * (kc + 1)],
                                                     start=(l == 0), stop=(l == KCH - 1))
                                nc.vector.tensor_mul(kT[h][:, 512 * kc:512 * (kc + 1)], ps[:, 0:512],
                                                     bc_rkv[:, 512 * kc:512 * (kc + 1)])
                        for kq in range(NKB // 4):
                            ps = pp.tile([128, 1024], f32, name=f"v_ps{kq}", tag="mm_ps", bufs=2)
                            for i in range(4):
                                kb = 4 * kq + i
                                for l in range(KCH):
                                    nc.tensor.matmul(ps[:, 256 * i:256 * (i + 1)],
                                                     ckv_g[l][:, 128 * kb:128 * (kb + 1)],
                                                     wkv_t[:, 256 * l:256 * (l + 1)],
                                                     start=(l == 0), stop=(l == KCH - 1))
                            for i in range(4):
                                kb = 4 * kq + i
                                nc.vector.tensor_scalar(out=v_t[kb][:], in0=ps[:, 256 * i:256 * (i + 1)],
                                                        scalar1=rkv_col[:, kb:kb + 1], scalar2=None,
                                                        op0=ALU.mult)

                    # rq row from the gathered q ssq
                    ssqq_g = tp.tile([1, S], dt, name="ssqq_g", tag="ssqg", bufs=1)
                    for r in range(NC_):
                        nc.sync.dma_start(out=ssqq_g[:, SL * r:SL * (r + 1)],
                                          in_=agr[r, SSQQ_ROW:SSQQ_ROW + 1, :])
                    mq = tp.tile([1, S], f32, name="mq", tag="mrow", bufs=1)
                    nc.scalar.activation(mq[:], ssqq_g[:], AF.Sqrt, scale=1.0 / QLR)
                    rq_row = tp.tile([1, S], f32, name="rq_row", tag="rrow", bufs=1)
                    nc.vector.reciprocal_approx_fast(out=rq_row[:], in_=mq[:])
                    rq_bf = rp.tile([1, S], dt, name="rq_bf")
                    nc.vector.tensor_copy(rq_bf[:], rq_row[:])

                    # ---- per panel: q projection then attention (both heads) ----
                    for p in range(NPANEL):
                        qs = slice(PANEL * p, PANEL * (p + 1))
                        bc_rq = tp.tile([128, PANEL], dt, name=f"bc_rq{p}", tag="bc_rq", bufs=2)
                        nc.gpsimd.partition_broadcast(bc_rq[:], rq_bf[0:1, qs])
                        qa_p = []
                        for l in range(QCH):
                            t = qap.tile([128, PANEL], dt, name=f"qa_p{p}_{l}", tag=f"qa_p{l}", bufs=2)
                            for r in range(2):
                                nc.sync.dma_start(out=t[:, SL * r:SL * (r + 1)],
                                                  in_=agr[2 * p + r, KV_ROWS + 128 * l:KV_ROWS + 128 * (l + 1), :])
                            qa_p.append(t)
                        for h in range(HPC):
                            hcol = 256 * h
                            ps_qnr = pp.tile([128, 1024], f32, name=f"qnr_ps{h}_{p}", tag="mm_ps", bufs=2)
                            for l in range(QCH):
                                nc.tensor.matmul(ps_qnr[:, 0:512], wqb_t[:, 512 * l + hcol:512 * l + hcol + NOPE],
                                                 qa_p[l][:], start=(l == 0), stop=(l == QCH - 1))
                            for l in range(QCH):
                                nc.tensor.matmul(ps_qnr[:, 512:1024], wqb_t[:, 512 * l + hcol + NOPE:512 * l + hcol + 256],
                                                 qa_p[l][:], start=(l == 0), stop=(l == QCH - 1))
                            qn = tp.tile([128, PANEL], dt, name=f"qn_sb{h}_{p}", tag="qn", bufs=4)
                            nc.vector.tensor_mul(qn[:], ps_qnr[:, 0:512], bc_rq[:])
                            qn_sb[(h, p)] = qn
                            qt1 = tp.tile([ROPE, PANEL], f32, name=f"qt1_{h}_{p}", tag="qt1", bufs=1)
                            nc.vector.tensor_mul(qt1[:], ps_qnr[0:ROPE, 512:1024], cos_g[:, qs])
                            qt2 = tp.tile([ROPE, PANEL], f32, name=f"qt2_{h}_{p}", tag="qt2", bufs=1)
                            nc.vector.tensor_mul(qt2[:], ps_qnr[ROPE:2 * ROPE, 512:1024], sin_g[:, qs])
                            qpt = tp.tile([ROPE, PANEL], f32, name=f"qpt_{h}_{p}", tag="qpt", bufs=1)
                            nc.vector.tensor_add(qpt[:], qt1[:], qt2[:])
                            qp = tp.tile([ROPE, PANEL], dt, name=f"qp_sb{h}_{p}", tag="qp", bufs=4)
                            nc.vector.tensor_mul(qp[:], qpt[:], bc_rq[0:ROPE, :])
                            qp_sb[(h, p)] = qp

                        for h in range(HPC):
                            nkb = 4 * (p + 1)
                            ps_at = ppat.tile([128, PANEL], f32, name=f"at_ps{h}_{p}", tag="at_ps", bufs=2)
                            acc = accp.tile([128, PANEL], f32, name=f"acc{h}_{p}", tag="acc", bufs=2)
                            pts = {}
                            pending = []

                            def consume(kb, nkb=nkb, acc=acc, ps_at=ps_at, pts=pts, h=h):
                                t, c0 = pts[kb]
                                nc.tensor.matmul(ps_at[:], v_t[kb][:, VD * h:VD * (h + 1)], t[:, c0:c0 + PANEL],
                                                 start=(kb == 0), stop=(kb == nkb - 1))
                                # softmax row-sum accumulates on the vector engine
                                if kb == 0:
                                    nc.vector.tensor_copy(acc[:], t[:, c0:c0 + PANEL])
                                else:
                                    nc.vector.tensor_add(acc[:], acc[:], t[:, c0:c0 + PANEL])

                            def flush():
                                while pending:
                                    consume(pending.pop(0))

                            kb = 0
                            while kb < nkb:
                                if kb + 1 < 4 * p:
                                    # two full blocks share one [128,1024] exp
                                    ps_sc = pp.tile([128, 1024], f32, name=f"sc2_{h}_{p}_{kb}", tag="mm_ps", bufs=2)
                                    for i in range(2):
                                        b = kb + i
                                        cs = slice(512 * i, 512 * (i + 1))
                                        nc.tensor.matmul(ps_sc[:, cs], kT[h][:, 128 * b:128 * (b + 1)],
                                                         qn_sb[(h, p)][:], start=True, stop=False)
                                        nc.tensor.matmul(ps_sc[:, cs], kpe_g[:, 128 * b:128 * (b + 1)],
                                                         qp_sb[(h, p)][:], start=False, stop=True)
                                    pt = ptp.tile([128, 1024], dt, name=f"pt{h}_{p}_{kb}", tag="pt", bufs=3)
                                    nc.scalar.activation(pt[:], ps_sc[:], AF.Exp, scale=SM_SCALE)
                                    pts[kb] = (pt, 0)
                                    pts[kb + 1] = (pt, 512)
                                    nxt = [kb, kb + 1]
                                    kb += 2
                                else:
                                    j = kb - 4 * p
                                    c0 = 128 * j if j > 0 else 0
                                    ps_sc = pp.tile([128, 1024], f32, name=f"sc1_{h}_{p}_{kb}", tag="mm_ps", bufs=2)
                                    nc.tensor.matmul(ps_sc[:, c0:PANEL], kT[h][:, 128 * kb:128 * (kb + 1)],
                                                     qn_sb[(h, p)][:, c0:PANEL], start=True, stop=False)
                                    nc.tensor.matmul(ps_sc[:, c0:PANEL], kpe_g[:, 128 * kb:128 * (kb + 1)],
                                                     qp_sb[(h, p)][:, c0:PANEL], start=False, stop=True)
                                    pt = ptp.tile([128, 1024], dt, name=f"pt{h}_{p}_{kb}", tag="pt", bufs=3)
                                    nc.scalar.activation(pt[:, c0:PANEL], ps_sc[:, c0:PANEL], AF.Exp, scale=SM_SCALE)
                                    if j >= 0:
                                        nc.gpsimd.affine_select(
                                            out=pt[:, 0:PANEL], in_=pt[:, 0:PANEL],
                                            pattern=[[1, PANEL]],
                                            compare_op=ALU.is_ge,
                                            fill=0.0,
                                            base=-128 * j,
                                            channel_multiplier=-1)
                                    pts[kb] = (pt, 0)
                                    nxt = [kb]
                                    kb += 1
                                flush()
                                pending.extend(nxt)
                            flush()

                            acc_bf = tp.tile([128, PANEL], dt, name=f"accb{h}_{p}", tag="accb", bufs=2)
                            nc.vector.tensor_copy(acc_bf[:], acc[:])
                            ps_sum = pp1.tile([1, PANEL], f32, name=f"sum_ps{h}_{p}", tag="sum_ps", bufs=2)
                            nc.tensor.matmul(ps_sum[:], ocol[:], acc_bf[:], start=True, stop=True)
                            rec = tp.tile([1, PANEL], f32, name=f"rec{h}_{p}", tag="rec", bufs=2)
                            nc.vector.reciprocal_approx_fast(out=rec[:], in_=ps_sum[:])
                            bc_sb = tp.tile([128, PANEL], f32, name=f"bc_sb{h}_{p}", tag="bc_sb", bufs=2)
                            nc.gpsimd.partition_broadcast(bc_sb[:], rec[:])
                            at_p = tp.tile([128, PANEL], dt, name=f"at_p{h}_{p}", tag="at_p", bufs=2)
                            nc.vector.tensor_mul(at_p[:], ps_at[:], bc_sb[:])
                            for r in range(2):
                                jdx = 2 * p + r
                                nc.scalar.dma_start(
                                    out=a2a_in[h][jdx * VD:(jdx + 1) * VD, :],
                                    in_=at_p[:, SL * r:SL * (r + 1)])

                            if p == NPANEL - 1 and h == 0:
                                nc.gpsimd.collective_compute(
                                    "AllToAll", ALU.bypass,
                                    replica_groups=[list(range(NC_))],
                                    ins=[a2a_in[0][:]], outs=[a2a_out[0][:]],
                                )
                                for jdx in range(NC_):
                                    c = 2 * jdx
                                    t = agp.tile([128, SL], dt, name=f"att_g{c}")
                                    nc.sync.dma_start(out=t[:], in_=a2a_out[0][128 * jdx:128 * (jdx + 1), :])
                                    att_g[c] = t
                                for par in range(2):
                                    for col in range(HID // 512):
                                        for c in range(par, HCH, 2):
                                            t = wsp.tile([128, 512], dt, name=f"wo_s{c}_{col}", tag="wo_s", bufs=24)
                                            nc.scalar.dma_start(
                                                out=t[:],
                                                in_=wo[:, HID * c + 512 * col:HID * c + 512 * (col + 1)])
                                            wo_s[(c, col)] = t

                    nc.gpsimd.collective_compute(
                        "AllToAll", ALU.bypass,
                        replica_groups=[list(range(NC_))],
                        ins=[a2a_in[1][:]], outs=[a2a_out[1][:]],
                    )
                    for jdx in range(NC_):
                        c = 2 * jdx + 1
                        t = agp.tile([128, SL], dt, name=f"att_g{c}")
                        nc.sync.dma_start(out=t[:], in_=a2a_out[1][128 * jdx:128 * (jdx + 1), :])
                        att_g[c] = t

                # ---- Wo in two halves: h0 during A2A-h1 flight, park PSUM ----
                with tc.tile_pool(name="wo_ps", bufs=1, space="PSUM") as wop, \
                     tc.tile_pool(name="wo_tmp", bufs=3) as wtp:
                    o_ps = {}
                    for col in range(HID // 512):
                        for qb in range(SL // 128):
                            ps = wop.tile([128, 512], f32, name=f"o_ps{col}_{qb}")
                            o_ps[(col, qb)] = ps
                            for c in range(0, HCH, 2):
                                nc.tensor.matmul(ps[:], att_g[c][:, 128 * qb:128 * (qb + 1)],
                                                 wo_s[(c, col)][:], start=(c == 0), stop=False)
                    for col in range(HID // 512):
                        for qb in range(SL // 128):
                            ps = o_ps[(col, qb)]
                            for c in range(1, HCH, 2):
                                nc.tensor.matmul(ps[:], att_g[c][:, 128 * qb:128 * (qb + 1)],
                                                 wo_s[(c, col)][:], start=False, stop=(c == HCH - 1))
                            osb = wtp.tile([128, 512], f32, name=f"osb{col}_{qb}", tag="osb", bufs=3)
                            nc.vector.tensor_copy(osb[:], ps[:])
                            nc.sync.dma_start(out=out_loc[128 * qb:128 * (qb + 1), 512 * col:512 * (col + 1)], in_=osb[:])

    nc.compile()
    return nc


def _to_dt(a, dt):
    if dt == bf16:
        return np.ascontiguousarray(a.astype(ml_dtypes.bfloat16))
    return np.ascontiguousarray(a.astype(np.float32))


def _pack_cols(mat, chunk_rows=128):
    """[R, C] -> [128, (R//128)*C]: row-chunk-major horizontal pack."""
    R, C = mat.shape
    n = R // chunk_rows
    return np.concatenate([mat[chunk_rows * i:chunk_rows * (i + 1), :] for i in range(n)], axis=1)


def _prepare_inputs(dt, hidden_states, position_ids, Wqa, qa_ln_w, Wqb, Wkva, kv_ln_w, Wkvb, Wo):
    perm = np.concatenate([np.arange(0, ROPE, 2), np.arange(1, ROPE, 2)])
    X = np.asarray(hidden_states, np.float32).reshape(S, HID)
    pos = np.asarray(position_ids).reshape(S).astype(np.float32)
    Wqa = np.asarray(Wqa, np.float32)
    Wkva = np.asarray(Wkva, np.float32)
    wqb_base = np.asarray(Wqb, np.float32) * np.asarray(qa_ln_w, np.float32)[:, None]
    wkvb_base = np.asarray(Wkvb, np.float32) * np.asarray(kv_ln_w, np.float32)[:, None]
    Wo = np.asarray(Wo, np.float32)

    inv = (1.0 / (THETA ** (np.arange(0, ROPE, 2, dtype=np.float32) / ROPE))).astype(np.float32)
    freqs = np.concatenate([inv, inv])                     # (64,)
    emb = pos[:, None] * freqs[None, :]                    # (S, 64)
    sin_all = np.ascontiguousarray(np.sin(emb).T.astype(np.float32))   # (64, S)
    cos_all = np.ascontiguousarray(np.cos(emb).T.astype(np.float32))

    wkva_kv = Wkva[:, :KVLR]
    wkva_pe = Wkva[:, KVLR:][:, perm]
    wa_kv = _pack_cols(np.concatenate([wkva_kv, wkva_pe], axis=1))     # [128, 16*576]
    wa_q = _pack_cols(Wqa)                                             # [128, 16*1536]

    head_blocks = []
    for h in range(NH):
        cols = wqb_base[:, 192 * h:192 * (h + 1)]
        nope = cols[:, :NOPE]
        pe_d = cols[:, NOPE:][:, perm]
        rot = np.concatenate([-pe_d[:, 32:], pe_d[:, :32]], axis=1)
        head_blocks.append(np.concatenate([nope, pe_d, rot], axis=1))  # (1536, 256)
    k_blocks = [wkvb_base[:, 256 * h:256 * h + NOPE] for h in range(NH)]
    v_blocks = [wkvb_base[:, 256 * h + NOPE:256 * (h + 1)] for h in range(NH)]

    wa_kv_d = _to_dt(wa_kv, dt)
    wa_q_d = _to_dt(wa_q, dt)
    wo_d = _to_dt(_pack_cols(Wo), dt)                                  # [128, 16*2048]
    ones_col_d = _to_dt(np.ones((128, 1), np.float32), dt)

    in_maps = []
    for c in range(NC_):
        rows = slice(SL * c, SL * (c + 1))
        wqb_core = np.concatenate([head_blocks[HPC * c + h] for h in range(HPC)], axis=1)   # (1536, 512)
        wkk_core = np.concatenate([k_blocks[HPC * c + h] for h in range(HPC)], axis=1)      # (512, 256)
        wkv_core = np.concatenate([v_blocks[HPC * c + h] for h in range(HPC)], axis=1)      # (512, 256)
        in_maps.append({
            "x_t": _to_dt(_pack_cols(np.ascontiguousarray(X[rows, :].T)), dt),
            "wa_kv": wa_kv_d,
            "wa_q": wa_q_d,
            "wqb": _to_dt(_pack_cols(wqb_core), dt),
            "wkvb_k": _to_dt(_pack_cols(wkk_core), dt),
            "wkvb_v": _to_dt(_pack_cols(wkv_core), dt),
            "wo": wo_d,
            "ones_col": ones_col_d,
            "sin_all": _to_dt(sin_all, dt),
            "cos_all": _to_dt(cos_all, dt),
            "sin_loc": _to_dt(sin_all[:, rows], dt),
            "cos_loc": _to_dt(cos_all[:, rows], dt),
        })
    return in_maps


def run(inputs, trace=False, trace_cores=None, dt=None):
    dt = dt if dt is not None else DT
    key = ("nc", str(dt))
    if key not in _CACHE:
        _CACHE[key] = build_program(dt)
    nc = _CACHE[key]
    in_maps = _prepare_inputs(dt, **inputs)
    res = run_bass_kernel_spmd(nc, in_maps, list(range(NC_)), trace=trace,
                               trace_cores=trace_cores)
    out = np.concatenate([res.results[c]["out_loc"] for c in range(NC_)], axis=0)
    return out.reshape(1, S, HID), res


def kernel(**inputs) -> np.ndarray:
    out, _ = run(inputs, trace=False)
    return out


# revision 25
# speedup vs baseline: 1.2275x; 1.0073x over previous
"""DeepseekV3 MLA flash-attention prefill kernel for 8 Trainium2 NeuronCores.

Sharding (SPMD, one program for all 8 cores):
  Stage A (sequence-parallel): core c computes low-rank down-projections
    X @ [Wqa|Wkva] for its 256 rows in transposed layout and AllGathers
    the UNNORMALIZED chunks plus the raw sum-of-squares rows (kv first,
    early trigger; q second). RMS scales are recovered cheaply in stage B
    (sqrt + fast-reciprocal) and folded into the K^T/V/q eviction
    multiplies, so nothing numeric sits on the AllGather trigger path.
  Stage B (head-parallel): core c owns heads {2c, 2c+1}: K^T/V built while
    the q AllGather is in flight; q projection and causal attention are
    interleaved per 512-wide panel; softmax runs in (k, q) layout without
    max subtraction; adjacent full k-blocks share one [128,1024] exp;
    diagonal blocks only compute/exp the live columns (affine_select
    zero-fills the rest); per-q normalization uses a fast DVE reciprocal
    plus a gpsimd partition-broadcast, double-buffered off critical path.
  Output: per-head AllToAll; Wo runs in two halves (h0 half during the
    h1 AllToAll flight, accumulating in parked PSUM banks).
"""

import sys

if '/opt/trn_rl_repo' not in sys.path:
    sys.path.insert(0, '/opt/trn_rl_repo')

import numpy as np
import ml_dtypes

import concourse.bass as bass
import concourse.mybir as mybir
import concourse.tile as tile
from concourse import bacc
from concourse.bass_utils import run_bass_kernel_spmd

f32 = mybir.dt.float32
f32r = mybir.dt.float32r
bf16 = mybir.dt.bfloat16
AF = mybir.ActivationFunctionType
ALU = mybir.AluOpType

NC_ = 8            # cores
S = 2048           # sequence
HID = 2048
QLR = 1536         # q lora rank
KVLR = 512         # kv lora rank
ROPE = 64
NOPE = 128
VD = 128
NH = 16
HPC = NH // NC_    # heads per core = 2
SL = S // NC_      # rows per core = 256
PANEL = 512        # q panel width
NPANEL = S // PANEL
NKB = S // 128     # 16 k blocks
QCH = QLR // 128   # 12
KCH = KVLR // 128  # 4
HCH = HID // 128   # 16
THETA = 10000.0
SM_SCALE = float((NOPE + ROPE) ** -0.5)

DT = bf16          # matmul dtype

NWG = 8            # wa_q DMA sub-groups (trickle arrival keeps HAM warm)
WARM1 = 24         # keep-warm links (4 chained mms each) during the AG flight

KV_ROWS = KCH * 128 + ROPE          # ckv | k_pe
AG_ROWS = KV_ROWS + QCH * 128 + 2   # + q_a | ssq_kv | ssq_q = 2114
SSQKV_ROW = AG_ROWS - 2
SSQQ_ROW = AG_ROWS - 1

_CACHE = {}


def build_program(dt):
    nc = bacc.Bacc("TRN2", target_bir_lowering=False, debug=False, num_devices=NC_)

    def din(name, shape, d=None):
        return nc.dram_tensor(name, shape, d or dt, kind="ExternalInput")

    x_t = din("x_t", [128, HCH * SL])                   # hc-major pack
    wa_kv = din("wa_kv", [128, HCH * (KVLR + ROPE)])    # per hc [kv|pe]
    wa_q = din("wa_q", [128, HCH * QLR])
    wqb = din("wqb", [128, QCH * HPC * 256])            # per l [nope|pe|rot]x2
    wkvb_k = din("wkvb_k", [128, KCH * HPC * NOPE])
    wkvb_v = din("wkvb_v", [128, KCH * HPC * VD])
    wo = din("wo", [128, HCH * HID])                    # per c rows pack
    ones_col = din("ones_col", [128, 1])
    sin_loc = din("sin_loc", [ROPE, SL])
    cos_loc = din("cos_loc", [ROPE, SL])
    sin_all = din("sin_all", [ROPE, S])
    cos_all = din("cos_all", [ROPE, S])
    out_loc = nc.dram_tensor("out_loc", [SL, HID], f32, kind="ExternalOutput")

    HG = HCH // 4          # hc per wa_kv/x group
    QHG = HCH // NWG       # hc per wa_q group

    with tile.TileContext(nc) as tc:
        with tc.tile_pool(name="dram", bufs=1, space="DRAM") as dpool, \
             tc.tile_pool(name="consts", bufs=1) as cpool:
            agw_in = dpool.tile([128, 1], dt)
            agw_out = dpool.tile([NC_ * 128, 1], dt, addr_space="Shared")
            ag_in = dpool.tile([AG_ROWS, SL], dt)
            ag_out = dpool.tile([NC_ * AG_ROWS, SL], dt, addr_space="Shared")
            a2a_in = [dpool.tile([NC_ * VD, SL], dt, name=f"a2a_in{h}") for h in range(HPC)]
            a2a_out = [dpool.tile([NC_ * VD, SL], dt, name=f"a2a_out{h}") for h in range(HPC)]

            ocol = cpool.tile([128, 1], dt)
            sin_l = cpool.tile([ROPE, SL], dt)
            cos_l = cpool.tile([ROPE, SL], dt)
            sin_g = cpool.tile([ROPE, S], dt)
            cos_g = cpool.tile([ROPE, S], dt)
            # stage-B weights hoisted to never-freed space so their loads
            # carry no WAR dependency on stage-A pools
            wkk_t = cpool.tile([128, KCH * HPC * NOPE], dt)
            wkv_t = cpool.tile([128, KCH * HPC * VD], dt)
            wqb_t = cpool.tile([128, QCH * HPC * 256], dt)
            gpboot = cpool.tile([1, 16], f32)

            # kick the gpsimd boot at t=0; warm-up collective absorbs the
            # CC-init + first-collective latency while stage A computes
            nc.gpsimd.memset(gpboot[:], 0.0)
            nc.scalar.dma_start(out=agw_in[:], in_=ones_col[:])
            nc.gpsimd.collective_compute(
                "AllGather", ALU.bypass,
                replica_groups=[list(range(NC_))],
                ins=[agw_in[:]], outs=[agw_out[:]],
            )

            nc.sync.dma_start(out=ocol[:], in_=ones_col[:])
            nc.scalar.dma_start(out=sin_l[:], in_=sin_loc[:])
            nc.scalar.dma_start(out=cos_l[:], in_=cos_loc[:])
            nc.scalar.dma_start(out=wkk_t[:], in_=wkvb_k[:])
            nc.scalar.dma_start(out=wkv_t[:], in_=wkvb_v[:])
            nc.scalar.dma_start(out=wqb_t[:], in_=wqb[:])
            nc.scalar.dma_start(out=sin_g[:], in_=sin_all[:])
            nc.scalar.dma_start(out=cos_g[:], in_=cos_all[:])

            # ================= Stage A: transposed down projections =================
            with tc.tile_pool(name="sa_x", bufs=1) as xp, \
                 tc.tile_pool(name="sa_w", bufs=1) as wp, \
                 tc.tile_pool(name="sa_tmp", bufs=2) as tp, \
                 tc.tile_pool(name="sa_ps", bufs=2, space="PSUM") as pp, \
                 tc.tile_pool(name="sa_ps1", bufs=1, space="PSUM") as pp1:

                x_g = []
                wakv_g = []
                for g in range(4):
                    xt = xp.tile([128, HG * SL], dt, name=f"xg{g}")
                    nc.sync.dma_start(out=xt[:], in_=x_t[:, HG * SL * g:HG * SL * (g + 1)])
                    x_g.append(xt)
                    wt = wp.tile([128, HG * 576], dt, name=f"wakv{g}")
                    nc.sync.dma_start(out=wt[:], in_=wa_kv[:, HG * 576 * g:HG * 576 * (g + 1)])
                    wakv_g.append(wt)
                waq_g = []
                for g in range(NWG):
                    wt = wp.tile([128, QHG * QLR], dt, name=f"waq{g}")
                    nc.sync.dma_start(out=wt[:], in_=wa_q[:, QHG * QLR * g:QHG * QLR * (g + 1)])
                    waq_g.append(wt)

                def xs(hc):
                    return x_g[hc // HG][:, SL * (hc % HG):SL * (hc % HG + 1)]

                # kv chunks: raw cast to AG staging + ssq accumulate
                ssq_kv = pp1.tile([1, SL], f32, name="ssq_kv")
                prev_sq = None
                for o in range(KCH):
                    ps = pp.tile([128, SL], f32, name=f"ps_kv{o}", tag="a_ps", bufs=2)
                    for hc in range(HCH):
                        w = wakv_g[hc // HG]
                        c0 = 576 * (hc % HG) + 128 * o
                        nc.tensor.matmul(ps[:], w[:, c0:c0 + 128], xs(hc),
                                         start=(hc == 0), stop=(hc == HCH - 1))
                    if prev_sq is not None:
                        nc.tensor.matmul(ssq_kv[:], ocol[:], prev_sq[:], start=(o == 1), stop=False)
                    cast = tp.tile([128, SL], dt, name=f"kvc{o}", tag="cast", bufs=3)
                    nc.vector.tensor_copy(cast[:], ps[:])
                    nc.scalar.dma_start(out=ag_in[128 * o:128 * (o + 1), :], in_=cast[:])
                    sq = tp.tile([128, SL], dt, name=f"sqk{o}", tag="sq", bufs=2)
                    nc.scalar.activation(sq[:], ps[:], AF.Square)
                    prev_sq = sq

                # pe chunk + rope (host sin/cos slices); k_pe is not rms-normed
                ps_pe = pp.tile([ROPE, SL], f32, name="ps_pe", tag="a_ps", bufs=2)
                for hc in range(HCH):
                    w = wakv_g[hc // HG]
                    c0 = 576 * (hc % HG) + KVLR
                    nc.tensor.matmul(ps_pe[:], w[:, c0:c0 + ROPE], xs(hc),
                                     start=(hc == 0), stop=(hc == HCH - 1))
                nc.tensor.matmul(ssq_kv[:], ocol[:], prev_sq[:], start=False, stop=True)
                krot = tp.tile([ROPE, SL], f32, name="krot", tag="krot", bufs=1)
                nc.vector.tensor_scalar(out=krot[0:32, :], in0=ps_pe[32:64, :], scalar1=-1.0, scalar2=None, op0=ALU.mult)
                nc.vector.tensor_copy(krot[32:64, :], ps_pe[0:32, :])
                kro = tp.tile([ROPE, SL], f32, name="kro", tag="kro", bufs=1)
                nc.vector.tensor_mul(kro[:], ps_pe[:], cos_l[:])
                krs = tp.tile([ROPE, SL], f32, name="krs", tag="krs", bufs=1)
                nc.vector.tensor_mul(krs[:], krot[:], sin_l[:])
                kfin = tp.tile([ROPE, SL], dt, name="kfin", tag="kfin", bufs=1)
                nc.vector.tensor_add(kfin[:], kro[:], krs[:])
                nc.scalar.dma_start(out=ag_in[KCH * 128:KCH * 128 + ROPE, :], in_=kfin[:])
                sqr_kv = tp.tile([1, SL], dt, name="sqr_kv", tag="sqr", bufs=2)
                nc.vector.tensor_copy(sqr_kv[:], ssq_kv[:])
                nc.scalar.dma_start(out=ag_in[SSQKV_ROW:SSQKV_ROW + 1, :], in_=sqr_kv[:])

                # q chunks (wa_q trickles in; ssq matmul lags one chunk)
                ssq_q = pp1.tile([1, SL], f32, name="ssq_q")
                prev_sq = None
                for o in range(QCH):
                    ps = pp.tile([128, SL], f32, name=f"ps_q{o}", tag="a_ps", bufs=2)
                    for hc in range(HCH):
                        w = waq_g[hc // QHG]
                        c0 = QLR * (hc % QHG) + 128 * o
                        nc.tensor.matmul(ps[:], w[:, c0:c0 + 128], xs(hc),
                                         start=(hc == 0), stop=(hc == HCH - 1))
                    if prev_sq is not None:
                        nc.tensor.matmul(ssq_q[:], ocol[:], prev_sq[:], start=(o == 1), stop=False)
                    cast = tp.tile([128, SL], dt, name=f"qc{o}", tag="cast", bufs=3)
                    nc.vector.tensor_copy(cast[:], ps[:])
                    nc.scalar.dma_start(out=ag_in[KV_ROWS + 128 * o:KV_ROWS + 128 * (o + 1), :], in_=cast[:])
                    sq = tp.tile([128, SL], dt, name=f"sqq{o}", tag="sq", bufs=2)
                    nc.scalar.activation(sq[:], ps[:], AF.Square)
                    prev_sq = sq
                nc.tensor.matmul(ssq_q[:], ocol[:], prev_sq[:], start=False, stop=True)
                sqr_q = tp.tile([1, SL], dt, name="sqr_q", tag="sqr", bufs=2)
                nc.vector.tensor_copy(sqr_q[:], ssq_q[:])
                nc.scalar.dma_start(out=ag_in[SSQQ_ROW:SSQQ_ROW + 1, :], in_=sqr_q[:])

                nc.gpsimd.collective_compute(
                    "AllGather", ALU.bypass,
                    replica_groups=[list(range(NC_))],
                    ins=[ag_in[:]], outs=[ag_out[:]],
                )

                # keep-warm chain through the AG flight, seeded off the
                # last q cast so it starts when stage-A compute drains
                warm_sb2 = tp.tile([128, 512], dt, name="warm_sb2", tag="warm2", bufs=1)
                nc.vector.tensor_copy(warm_sb2[:, 0:256], cast[:])
                nc.vector.tensor_copy(warm_sb2[:, 256:512], cast[:])
                warm_ps2 = pp1.tile([1, 512], f32, name="warm_ps2")
                wfb2 = tp.tile([1, 16], f32, name="wfb2", tag="wfb2", bufs=1)
                for i in range(WARM1):
                    for j4 in range(4):
                        nc.tensor.matmul(warm_ps2[:], ocol[:], warm_sb2[:], start=(j4 == 0), stop=(j4 == 3))
                    nc.vector.tensor_copy(wfb2[:], warm_ps2[:, 0:16])
                    nc.vector.tensor_copy(warm_sb2[0:1, 0:16], wfb2[:])

            agr = ag_out.rearrange("(r c) q -> r c q", r=NC_)

            # ================= Stage B: head-parallel attention =================
            with tc.tile_pool(name="sb_res", bufs=1) as rp, \
                 tc.tile_pool(name="sb_wo", bufs=24) as wsp, \
                 tc.tile_pool(name="sb_ag", bufs=1) as agp:
                att_g = {}
                wo_s = {}
                qn_sb = {}
                qp_sb = {}

                with tc.tile_pool(name="sb_qa", bufs=2) as qap, \
                     tc.tile_pool(name="sb_tmp", bufs=2) as tp, \
                     tc.tile_pool(name="sb_acc", bufs=2) as accp, \
                     tc.tile_pool(name="sb_pt", bufs=3) as ptp, \
                     tc.tile_pool(name="sb_ps", bufs=2, space="PSUM") as pp, \
                     tc.tile_pool(name="sb_psat", bufs=2, space="PSUM") as ppat, \
                     tc.tile_pool(name="sb_ps1", bufs=1, space="PSUM") as pp1:

                    # gathered kv rows (sync queue)
                    kpe_g = rp.tile([ROPE, S], dt, name="kpe_g")
                    ssqkv_g = tp.tile([1, S], dt, name="ssqkv_g", tag="ssqg", bufs=1)
                    nc.sync.dma_start(
                        out=kpe_g[:].rearrange("c (r q) -> c r q", r=NC_),
                        in_=agr[:, KCH * 128:KCH * 128 + ROPE, :].rearrange("r c q -> c r q"))
                    nc.sync.dma_start(
                        out=ssqkv_g[:].rearrange("c (r q) -> c r q", r=NC_),
                        in_=agr[:, SSQKV_ROW:SSQKV_ROW + 1, :].rearrange("r c q -> c r q"))
                    # rkv row: sqrt(ssq/512) then fast reciprocal
                    mkv = tp.tile([1, S], f32, name="mkv", tag="mrow", bufs=1)
                    nc.scalar.activation(mkv[:], ssqkv_g[:], AF.Sqrt, scale=1.0 / KVLR)
                    rkv_row = tp.tile([1, S], f32, name="rkv_row", tag="rrow", bufs=1)
                    nc.vector.reciprocal_approx_fast(out=rkv_row[:], in_=mkv[:])
                    rkv_bf = rp.tile([1, S], dt, name="rkv_bf")  # kept: feeds PE transposes
                    nc.vector.tensor_copy(rkv_bf[:], rkv_row[:])
                    bc_rkv = rp.tile([128, S], dt, name="bc_rkv")
                    nc.gpsimd.partition_broadcast(bc_rkv[:], rkv_bf[:])
                    # rkv as columns for the V scale: 16x [1,128]->[128,1] via PE
                    ps_col = pp.tile([128, 1024], f32, name="ps_col", tag="mm_ps", bufs=2)
                    for kb in range(NKB):
                        nc.tensor.matmul(ps_col[:, kb:kb + 1], rkv_bf[0:1, 128 * kb:128 * (kb + 1)],
                                         ocol[0:1, 0:1], start=True, stop=True)
                    rkv_col = rp.tile([128, NKB], f32, name="rkv_col")
                    nc.vector.tensor_copy(rkv_col[:], ps_col[:, 0:NKB])

                    kT = [rp.tile([128, S], dt, name=f"kT{h}") for h in range(HPC)]
                    v_t = [rp.tile([128, HPC * VD], dt, name=f"v_t{kb}") for kb in range(NKB)]
                    with tc.tile_pool(name="sb_ckv", bufs=1) as ckvp:
                        ckv_g = []
                        for j in range(KCH):
                            t = ckvp.tile([128, S], dt, name=f"ckv_g{j}")
                            nc.sync.dma_start(
                                out=t[:].rearrange("c (r q) -> c r q", r=NC_),
                                in_=agr[:, 128 * j:128 * (j + 1), :].rearrange("r c q -> c r q"))
                            ckv_g.append(t)
                        for h in range(HPC):
                            for kc in range(S // 512):
                                ps = pp.tile([128, 1024], f32, name=f"kt_ps{h}_{kc}", tag="mm_ps", bufs=2)
                                for l in range(KCH):
                                    nc.tensor.matmul(ps[:, 0:512],
                                                     wkk_t[:, 256 * l + NOPE * h:256 * l + NOPE * (h + 1)],
                                                     ckv_g[l][:, 512 * kc:512 * (kc + 1)],
                                                     start=(l == 0), stop=(l == KCH - 1))
                                nc.vector.tensor_mul(kT[h][:, 512 * kc:512 * (kc + 1)], ps[:, 0:512],
                                                     bc_rkv[:, 512 * kc:512 * (kc + 1)])
                        for kq in range(NKB // 4):
                            ps = pp.tile([128, 1024], f32, name=f"v_ps{kq}", tag="mm_ps", bufs=2)
                            for i in range(4):
                                kb = 4 * kq + i
                                for l in range(KCH):
                                    nc.tensor.matmul(ps[:, 256 * i:256 * (i + 1)],
                                                     ckv_g[l][:, 128 * kb:128 * (kb + 1)],
                                                     wkv_t[:, 256 * l:256 * (l + 1)],
                                                     start=(l == 0), stop=(l == KCH - 1))
                            for i in range(4):
                                kb = 4 * kq + i
                                nc.vector.tensor_scalar(out=v_t[kb][:], in0=ps[:, 256 * i:256 * (i + 1)],
                                                        scalar1=rkv_col[:, kb:kb + 1], scalar2=None,
                                                        op0=ALU.mult)

                    # rq row from the gathered q ssq
                    ssqq_g = tp.tile([1, S], dt, name="ssqq_g", tag="ssqg", bufs=1)
                    nc.sync.dma_start(
                        out=ssqq_g[:].rearrange("c (r q) -> c r q", r=NC_),
                        in_=agr[:, SSQQ_ROW:SSQQ_ROW + 1, :].rearrange("r c q -> c r q"))
                    mq = tp.tile([1, S], f32, name="mq", tag="mrow", bufs=1)
                    nc.scalar.activation(mq[:], ssqq_g[:], AF.Sqrt, scale=1.0 / QLR)
                    rq_row = tp.tile([1, S], f32, name="rq_row", tag="rrow", bufs=1)
                    nc.vector.reciprocal_approx_fast(out=rq_row[:], in_=mq[:])
                    rq_bf = rp.tile([1, S], dt, name="rq_bf")
                    nc.vector.tensor_copy(rq_bf[:], rq_row[:])

                    # ---- per panel: q projection then attention (both heads) ----
                    for p in range(NPANEL):
                        qs = slice(PANEL * p, PANEL * (p + 1))
                        bc_rq = tp.tile([128, PANEL], dt, name=f"bc_rq{p}", tag="bc_rq", bufs=2)
                        nc.gpsimd.partition_broadcast(bc_rq[:], rq_bf[0:1, qs])
                        qa_p = []
                        for l in range(QCH):
                            t = qap.tile([128, PANEL], dt, name=f"qa_p{p}_{l}", tag=f"qa_p{l}", bufs=2)
                            nc.sync.dma_start(
                                out=t[:].rearrange("c (r q) -> c r q", r=2),
                                in_=agr[2 * p:2 * p + 2, KV_ROWS + 128 * l:KV_ROWS + 128 * (l + 1), :].rearrange("r c q -> c r q"))
                            qa_p.append(t)
                        for h in range(HPC):
                            hcol = 256 * h
                            ps_qnr = pp.tile([128, 1024], f32, name=f"qnr_ps{h}_{p}", tag="mm_ps", bufs=2)
                            for l in range(QCH):
                                nc.tensor.matmul(ps_qnr[:, 0:512], wqb_t[:, 512 * l + hcol:512 * l + hcol + NOPE],
                                                 qa_p[l][:], start=(l == 0), stop=(l == QCH - 1))
                            for l in range(QCH):
                                nc.tensor.matmul(ps_qnr[:, 512:1024], wqb_t[:, 512 * l + hcol + NOPE:512 * l + hcol + 256],
                                                 qa_p[l][:], start=(l == 0), stop=(l == QCH - 1))
                            qn = tp.tile([128, PANEL], dt, name=f"qn_sb{h}_{p}", tag="qn", bufs=4)
                            nc.vector.tensor_mul(qn[:], ps_qnr[:, 0:512], bc_rq[:])
                            qn_sb[(h, p)] = qn
                            qt1 = tp.tile([ROPE, PANEL], f32, name=f"qt1_{h}_{p}", tag="qt1", bufs=1)
                            nc.vector.tensor_mul(qt1[:], ps_qnr[0:ROPE, 512:1024], cos_g[:, qs])
                            qt2 = tp.tile([ROPE, PANEL], f32, name=f"qt2_{h}_{p}", tag="qt2", bufs=1)
                            nc.vector.tensor_mul(qt2[:], ps_qnr[ROPE:2 * ROPE, 512:1024], sin_g[:, qs])
                            qpt = tp.tile([ROPE, PANEL], f32, name=f"qpt_{h}_{p}", tag="qpt", bufs=1)
                            nc.vector.tensor_add(qpt[:], qt1[:], qt2[:])
                            qp = tp.tile([ROPE, PANEL], dt, name=f"qp_sb{h}_{p}", tag="qp", bufs=4)
                            nc.vector.tensor_mul(qp[:], qpt[:], bc_rq[0:ROPE, :])
                            qp_sb[(h, p)] = qp

                        for h in range(HPC):
                            nkb = 4 * (p + 1)
                            ps_at = ppat.tile([128, PANEL], f32, name=f"at_ps{h}_{p}", tag="at_ps", bufs=2)
                            acc = accp.tile([128, PANEL], f32, name=f"acc{h}_{p}", tag="acc", bufs=2)
                            pts = {}
                            pending = []

                            def consume(kb, nkb=nkb, acc=acc, ps_at=ps_at, pts=pts, h=h):
                                t, c0 = pts[kb]
                                nc.tensor.matmul(ps_at[:], v_t[kb][:, VD * h:VD * (h + 1)], t[:, c0:c0 + PANEL],
                                                 start=(kb == 0), stop=(kb == nkb - 1))
                                # softmax row-sum accumulates on the vector engine
                                if kb == 0:
                                    nc.vector.tensor_copy(acc[:], t[:, c0:c0 + PANEL])
                                else:
                                    nc.vector.tensor_add(acc[:], acc[:], t[:, c0:c0 + PANEL])

                            def flush():
                                while pending:
                                    consume(pending.pop(0))

                            kb = 0
                            while kb < nkb:
                                if kb + 1 < 4 * p:
                                    # two full blocks share one [128,1024] exp
                                    ps_sc = pp.tile([128, 1024], f32, name=f"sc2_{h}_{p}_{kb}", tag="mm_ps", bufs=2)
                                    for i in range(2):
                                        b = kb + i
                                        cs = slice(512 * i, 512 * (i + 1))
                                        nc.tensor.matmul(ps_sc[:, cs], kT[h][:, 128 * b:128 * (b + 1)],
                                                         qn_sb[(h, p)][:], start=True, stop=False)
                                        nc.tensor.matmul(ps_sc[:, cs], kpe_g[:, 128 * b:128 * (b + 1)],
                                                         qp_sb[(h, p)][:], start=False, stop=True)
                                    pt = ptp.tile([128, 1024], dt, name=f"pt{h}_{p}_{kb}", tag="pt", bufs=3)
                                    nc.scalar.activation(pt[:], ps_sc[:], AF.Exp, scale=SM_SCALE)
                                    pts[kb] = (pt, 0)
                                    pts[kb + 1] = (pt, 512)
                                    nxt = [kb, kb + 1]
                                    kb += 2
                                else:
                                    j = kb - 4 * p
                                    c0 = 128 * j if j > 0 else 0
                                    ps_sc = pp.tile([128, 1024], f32, name=f"sc1_{h}_{p}_{kb}", tag="mm_ps", bufs=2)
                                    nc.tensor.matmul(ps_sc[:, c0:PANEL], kT[h][:, 128 * kb:128 * (kb + 1)],
                                                     qn_sb[(h, p)][:, c0:PANEL], start=True, stop=False)
                                    nc.tensor.matmul(ps_sc[:, c0:PANEL], kpe_g[:, 128 * kb:128 * (kb + 1)],
                                                     qp_sb[(h, p)][:, c0:PANEL], start=False, stop=True)
                                    pt = ptp.tile([128, 1024], dt, name=f"pt{h}_{p}_{kb}", tag="pt", bufs=3)
                                    nc.scalar.activation(pt[:, c0:PANEL], ps_sc[:, c0:PANEL], AF.Exp, scale=SM_SCALE)
                                    if j >= 0:
                                        nc.gpsimd.affine_select(
                                            out=pt[:, 0:PANEL], in_=pt[:, 0:PANEL],
                                            pattern=[[1, PANEL]],
                                            compare_op=ALU.is_ge,
                                            fill=0.0,
                                            base=-128 * j,
                                            channel_multiplier=-1)
                                    pts[kb] = (pt, 0)
                                    nxt = [kb]
                                    kb += 1
                                flush()
                                pending.extend(nxt)
                            flush()

                            acc_bf = tp.tile([128, PANEL], dt, name=f"accb{h}_{p}", tag="accb", bufs=2)
                            nc.vector.tensor_copy(acc_bf[:], acc[:])
                            ps_sum = pp1.tile([1, PANEL], f32, name=f"sum_ps{h}_{p}", tag="sum_ps", bufs=2)
                            nc.tensor.matmul(ps_sum[:], ocol[:], acc_bf[:], start=True, stop=True)
                            rec = tp.tile([1, PANEL], f32, name=f"rec{h}_{p}", tag="rec", bufs=2)
                            nc.vector.reciprocal_approx_fast(out=rec[:], in_=ps_sum[:])
                            bc_sb = tp.tile([128, PANEL], f32, name=f"bc_sb{h}_{p}", tag="bc_sb", bufs=2)
                            nc.gpsimd.partition_broadcast(bc_sb[:], rec[:])
                            at_p = tp.tile([128, PANEL], dt, name=f"at_p{h}_{p}", tag="at_p", bufs=2)
                            nc.vector.tensor_mul(at_p[:], ps_at[:], bc_sb[:])
                            for r in range(2):
                                jdx = 2 * p + r
                                nc.scalar.dma_start(
                                    out=a2a_in[h][jdx * VD:(jdx + 1) * VD, :],
                                    in_=at_p[:, SL * r:SL * (r + 1)])

                            if p == NPANEL - 1 and h == 0:
                                nc.gpsimd.collective_compute(
                                    "AllToAll", ALU.bypass,
                                    replica_groups=[list(range(NC_))],
                                    ins=[a2a_in[0][:]], outs=[a2a_out[0][:]],
                                )
                                att_all0 = agp.tile([128, S], dt, name="att_all0")
                                nc.sync.dma_start(
                                    out=att_all0[:].rearrange("c (r q) -> c r q", r=NC_),
                                    in_=a2a_out[0].rearrange("(r c) q -> r c q", r=NC_).rearrange("r c q -> c r q"))
                                att_g[0] = att_all0
                                for par in range(2):
                                    for col in range(HID // 512):
                                        for c in range(par, HCH, 2):
                                            t = wsp.tile([128, 512], dt, name=f"wo_s{c}_{col}", tag="wo_s", bufs=24)
                                            nc.scalar.dma_start(
                                                out=t[:],
                                                in_=wo[:, HID * c + 512 * col:HID * c + 512 * (col + 1)])
                                            wo_s[(c, col)] = t

                    nc.gpsimd.collective_compute(
                        "AllToAll", ALU.bypass,
                        replica_groups=[list(range(NC_))],
                        ins=[a2a_in[1][:]], outs=[a2a_out[1][:]],
                    )
                    att_all1 = agp.tile([128, S], dt, name="att_all1")
                    nc.sync.dma_start(
                        out=att_all1[:].rearrange("c (r q) -> c r q", r=NC_),
                        in_=a2a_out[1].rearrange("(r c) q -> r c q", r=NC_).rearrange("r c q -> c r q"))
                    att_g[1] = att_all1

                # ---- Wo in two halves: h0 during A2A-h1 flight, park PSUM ----
                with tc.tile_pool(name="wo_ps", bufs=1, space="PSUM") as wop, \
                     tc.tile_pool(name="wo_tmp", bufs=3) as wtp:
                    o_ps = {}
                    for col in range(HID // 512):
                        for qb in range(SL // 128):
                            ps = wop.tile([128, 512], f32, name=f"o_ps{col}_{qb}")
                            o_ps[(col, qb)] = ps
                            for c in range(0, HCH, 2):
                                lhs = att_g[0][:, SL * (c // 2) + 128 * qb:SL * (c // 2) + 128 * (qb + 1)]
                                nc.tensor.matmul(ps[:], lhs, wo_s[(c, col)][:], start=(c == 0), stop=False)
                    for col in range(HID // 512):
                        for qb in range(SL // 128):
                            ps = o_ps[(col, qb)]
                            for c in range(1, HCH, 2):
                                lhs = att_g[1][:, SL * (c // 2) + 128 * qb:SL * (c // 2) + 128 * (qb + 1)]
                                nc.tensor.matmul(ps[:], lhs, wo_s[(c, col)][:], start=False, stop=(c == HCH - 1))
                            osb = wtp.tile([128, 512], f32, name=f"osb{col}_{qb}", tag="osb", bufs=3)
                            nc.vector.tensor_copy(osb[:], ps[:])
                            nc.scalar.dma_start(out=out_loc[128 * qb:128 * (qb + 1), 512 * col:512 * (col + 1)], in_=osb[:])

    nc.compile()
    return nc


def _to_dt(a, dt):
    if dt == bf16:
        return np.ascontiguousarray(a.astype(ml_dtypes.bfloat16))
    return np.ascontiguousarray(a.astype(np.float32))


def _pack_cols(mat, chunk_rows=128):
    """[R, C] -> [128, (R//128)*C]: row-chunk-major horizontal pack."""
    R, C = mat.shape
    n = R // chunk_rows
    return np.concatenate([mat[chunk_rows * i:chunk_rows * (i + 1), :] for i in range(n)], axis=1)


def _prepare_inputs(dt, hidden_states, position_ids, Wqa, qa_ln_w, Wqb, Wkva, kv_ln_w, Wkvb, Wo):
    perm = np.concatenate([np.arange(0, ROPE, 2), np.arange(1, ROPE, 2)])
    X = np.asarray(hidden_states, np.float32).reshape(S, HID)
    pos = np.asarray(position_ids).reshape(S).astype(np.float32)
    Wqa = np.asarray(Wqa, np.float32)
    Wkva = np.asarray(Wkva, np.float32)
    wqb_base = np.asarray(Wqb, np.float32) * np.asarray(qa_ln_w, np.float32)[:, None]
    wkvb_base = np.asarray(Wkvb, np.float32) * np.asarray(kv_ln_w, np.float32)[:, None]
    Wo = np.asarray(Wo, np.float32)

    inv = (1.0 / (THETA ** (np.arange(0, ROPE, 2, dtype=np.float32) / ROPE))).astype(np.float32)
    freqs = np.concatenate([inv, inv])                     # (64,)
    emb = pos[:, None] * freqs[None, :]                    # (S, 64)
    sin_all = np.ascontiguousarray(np.sin(emb).T.astype(np.float32))   # (64, S)
    cos_all = np.ascontiguousarray(np.cos(emb).T.astype(np.float32))

    wkva_kv = Wkva[:, :KVLR]
    wkva_pe = Wkva[:, KVLR:][:, perm]
    wa_kv = _pack_cols(np.concatenate([wkva_kv, wkva_pe], axis=1))     # [128, 16*576]
    wa_q = _pack_cols(Wqa)                                             # [128, 16*1536]

    head_blocks = []
    for h in range(NH):
        cols = wqb_base[:, 192 * h:192 * (h + 1)]
        nope = cols[:, :NOPE]
        pe_d = cols[:, NOPE:][:, perm]
        rot = np.concatenate([-pe_d[:, 32:], pe_d[:, :32]], axis=1)
        head_blocks.append(np.concatenate([nope, pe_d, rot], axis=1))  # (1536, 256)
    k_blocks = [wkvb_base[:, 256 * h:256 * h + NOPE] for h in range(NH)]
    v_blocks = [wkvb_base[:, 256 * h + NOPE:256 * (h + 1)] for h in range(NH)]

    wa_kv_d = _to_dt(wa_kv, dt)
    wa_q_d = _to_dt(wa_q, dt)
    wo_d = _to_dt(_pack_cols(Wo), dt)                                  # [128, 16*2048]
    ones_col_d = _to_dt(np.ones((128, 1), np.float32), dt)

    in_maps = []
    for c in range(NC_):
        rows = slice(SL * c, SL * (c + 1))
        wqb_core = np.concatenate([head_blocks[HPC * c + h] for h in range(HPC)], axis=1)   # (1536, 512)
        wkk_core = np.concatenate([k_blocks[HPC * c + h] for h in range(HPC)], axis=1)      # (512, 256)
        wkv_core = np.concatenate([v_blocks[HPC * c + h] for h in range(HPC)], axis=1)      # (512, 256)
        in_maps.append({
            "x_t": _to_dt(_pack_cols(np.ascontiguousarray(X[rows, :].T)), dt),
            "wa_kv": wa_kv_d,
            "wa_q": wa_q_d,
            "wqb": _to_dt(_pack_cols(wqb_core), dt),
            "wkvb_k": _to_dt(_pack_cols(wkk_core), dt),
            "wkvb_v": _to_dt(_pack_cols(wkv_core), dt),
            "wo": wo_d,
            "ones_col": ones_col_d,
            "sin_all": _to_dt(sin_all, dt),
            "cos_all": _to_dt(cos_all, dt),
            "sin_loc": _to_dt(sin_all[:, rows], dt),
            "cos_loc": _to_dt(cos_all[:, rows], dt),
        })
    return in_maps


def run(inputs, trace=False, trace_cores=None, dt=None):
    dt = dt if dt is not None else DT
    key = ("nc", str(dt))
    if key not in _CACHE:
        _CACHE[key] = build_program(dt)
    nc = _CACHE[key]
    in_maps = _prepare_inputs(dt, **inputs)
    res = run_bass_kernel_spmd(nc, in_maps, list(range(NC_)), trace=trace,
                               trace_cores=trace_cores)
    out = np.concatenate([res.results[c]["out_loc"] for c in range(NC_)], axis=0)
    return out.reshape(1, S, HID), res


def kernel(**inputs) -> np.ndarray:
    out, _ = run(inputs, trace=False)
    return out
